# revision 1
# baseline (speedup 1.0000x reference)
"""Causal self-attention (B=1, S=4096, D=768, H=12, dh=64) on 8 TRN2 NeuronCores.

Strategy:
  - Sequence-parallel QKV projections + RoPE (each core projects 512 rows).
  - K/V (bf16; V carries a ones-column for the softmax denominator) are
    projected, rope'd and bounced out one 128-row quarter at a time, each
    quarter feeding its own AllGather so gathers overlap later projection
    work; the Q projection is emitted last to overlap the gathers too.
  - Attention is query-sharded with a stride-8 interleave (core c owns query
    rows c::8) so causal work is balanced and the program is SPMD-uniform;
    all per-core variation is input data (x slices, rope tables, masks).
  - KV rows are owned in interleaved 64-row blocks (block b -> core b%8) so the
    gather splits into 4 pipelined quarter-AllGathers, each delivering key
    chunks 8u..8u+7 in causal consumption order; gather + K/V reload overlap
    attention on the early chunks.
  - Transposed layout throughout: S^T = K^T.T @ Q^T has keys on partitions, so
    the softmax sum falls out of the AV matmul via the ones-row of V.
  - QK runs two heads concurrently via PE row-groups (0,*)/(64,*) with separate
    PSUM banks (3-chunk groups: 2x3 st banks + 2 ot accumulators = 8 banks).
  - Causal mask = per-128-key-chunk band multiply on a [128,3,48] window
    (band position within a group is core-independent; mask values are data).
  - RoPE: rot = A*cos + swap(A)*sin_signed with the sign in the host-built
    sin table; for K the swap is 4 quadrant-aligned DVE copies straight out
    of PSUM (keeps the AllGather-feeding chain short), for Q it is SBUF DMAs
    off a Scalar-engine PSUM copy. Logit scale is folded into exp().
"""

import numpy as np
import ml_dtypes

import concourse.bass as bass
import concourse.bacc as bacc
import concourse.tile as tile
import concourse.mybir as mybir
import concourse.bass_utils as bass_utils

NCORES = 8
S = 4096
D = 768
H = 12
DH = 64
HALF = 32
P = 128
SL = S // NCORES          # 512 local queries / kv rows per core
KSUB = D // P             # 6
NKC = S // P              # 32 key chunks of 128
GK = 4                    # key chunks per exp group
NG = NKC // GK            # 8 groups
KS = D * SL               # K^T slice elems (768*512)
VW = H * (DH + 1)         # 780: V row width incl. ones col per head
VS = SL * VW              # V slice elems
NQ = 4                    # pipelined AllGather quarters
KQ = D * P                # K^T part per quarter (768*128)
VQ = P * VW               # V part per quarter
RQ = KQ + VQ              # per-rank elems per quarter
F32 = mybir.dt.float32
BF16 = mybir.dt.bfloat16

_cache = {}


def _build(repeats=1, fake_gather=False, stop_after=None):
    nc = bacc.Bacc(
        "TRN2",
        target_bir_lowering=False,
        debug=False,
        enable_asserts=False,
        num_devices=1 if fake_gather else NCORES,
    )
    inp = {}
    for name, shape, dt in [
        ("xq", [D, SL], BF16),
        ("xkv", [D, SL], BF16),
        ("cosq", [P, SL], BF16),
        ("sinq", [P, SL], BF16),
        ("cosk", [P, SL], BF16),
        ("sink", [P, SL], BF16),
        ("mask3", [P, 3, 48], BF16),
        ("wq", [D, D], BF16),
        ("wk", [D, D], BF16),
        ("wv", [D, D], BF16),
        ("wo", [D, D], BF16),
    ]:
        inp[name] = nc.dram_tensor(name, shape, dt, kind="ExternalInput")
    out_d = nc.dram_tensor("out", [KSUB, P, SL], F32, kind="ExternalOutput")

    with tile.TileContext(nc) as tc:
      for _rep in range(repeats):
        with (
            tc.tile_pool(name="persist", bufs=1) as persist,
            tc.tile_pool(name="dram", bufs=1, space="DRAM") as dram,
        ):
            # ---- persistent tiles ----
            qrot_t = [
                persist.tile([P, SL], BF16, name=f"qrot{s_}", tag=f"qrot{s_}")
                for s_ in range(KSUB)
            ]
            osb = persist.tile([64, H, SL], BF16)
            mask_sb = persist.tile([P, 3, 48], BF16)
            nc.sync.dma_start(mask_sb[:], inp["mask3"].ap())
            wo_sb = persist.tile([64, H, D], BF16)
            for h in range(H):
                nc.sync.dma_start(
                    wo_sb[:, h, :],
                    inp["wo"].ap().rearrange("(h p) e -> p h e", p=64)[:, h, :],
                )

            kvin = dram.tile([NQ, RQ], BF16)
            kvout = [
                dram.tile(
                    [NCORES, RQ],
                    BF16,
                    name=f"kvout{u}",
                    addr_space="Local" if fake_gather else "Shared",
                )
                for u in range(NQ)
            ]

            # ================= Phase A: projections + rope =================
            with (
                tc.tile_pool(name="pw", bufs=1) as pw,
                tc.tile_pool(name="px", bufs=1) as px,
                tc.tile_pool(name="pt", bufs=3) as pt,
                tc.tile_pool(name="psA", bufs=2, space="PSUM") as psA,
            ):
                w_sb = {}
                for name in ["wq", "wk", "wv"]:
                    w_sb[name] = pw.tile([P, KSUB, D], BF16, name=f"{name}_sb")
                    for ks in range(KSUB):
                        nc.sync.dma_start(
                            w_sb[name][:, ks, :],
                            inp[name].ap().rearrange("(ks p) m -> p ks m", p=P)[
                                :, ks, :
                            ],
                        )
                xq_sb = px.tile([P, KSUB, SL], BF16)
                xkv_sb = px.tile([P, KSUB, SL], BF16)
                for ks in range(KSUB):
                    nc.sync.dma_start(
                        xq_sb[:, ks, :],
                        inp["xq"].ap().rearrange("(ks p) n -> p ks n", p=P)[:, ks, :],
                    )
                    nc.sync.dma_start(
                        xkv_sb[:, ks, :],
                        inp["xkv"].ap().rearrange("(ks p) n -> p ks n", p=P)[:, ks, :],
                    )
                trig = {}
                for name in ["cosq", "sinq", "cosk", "sink"]:
                    trig[name] = px.tile([P, SL], BF16, name=f"{name}_sb")
                    nc.sync.dma_start(trig[name][:], inp[name].ap())

                vloc = px.tile([P, S // P // NCORES, H, DH + 1], BF16)

                def project_rope(wname, x_sb, cos_t, sin_t, dest, subs):
                    # sin_t is block-signed: rows 0-31=-sin, 32-63=+sin, etc.
                    for s in subs:
                        pa = psA.tile([P, SL], F32, name="pa", tag="pa")
                        for ks in range(KSUB):
                            nc.tensor.matmul(
                                pa[:],
                                lhsT=w_sb[wname][:, ks, s * P : (s + 1) * P],
                                rhs=x_sb[:, ks, :],
                                start=(ks == 0),
                                stop=(ks == KSUB - 1),
                            )
                        pab = pt.tile([P, SL], BF16, name="pab", tag="pab")
                        nc.scalar.copy(pab[:], pa[:])
                        swp = pt.tile([P, SL], BF16, name="swp", tag="swp")
                        for (dd, ss2) in [(0, 32), (32, 0), (64, 96), (96, 64)]:
                            nc.sync.dma_start(
                                swp[dd : dd + 32, :], pab[ss2 : ss2 + 32, :]
                            )
                        t1 = pt.tile([P, SL], BF16, name="t1", tag="t1")
                        t2 = pt.tile([P, SL], BF16, name="t2", tag="t2")
                        nc.vector.tensor_mul(t1[:], pab[:], cos_t[:])
                        nc.vector.tensor_mul(t2[:], swp[:], sin_t[:])
                        nc.vector.tensor_add(dest[s][:], t1[:], t2[:])

                # K + V projection, rope and bounce-out one QUARTER (128 kv
                # rows) at a time so each quarter-AllGather launches as soon
                # as its data exists, overlapping later projection work and
                # attention on early key chunks.
                nc.vector.memset(vloc[:, :, :, DH : DH + 1], 1.0)
                for u in range(NQ):
                    kq = pt.tile([P, KSUB, P], BF16, name="kq", tag="kq")
                    for s in range(KSUB):
                        pa = psA.tile([P, P], F32, name="pak", tag="pak")
                        for ks in range(KSUB):
                            nc.tensor.matmul(
                                pa[:],
                                lhsT=w_sb["wk"][:, ks, s * P : (s + 1) * P],
                                rhs=xkv_sb[:, ks, u * P : (u + 1) * P],
                                start=(ks == 0),
                                stop=(ks == KSUB - 1),
                            )
                        swp = pt.tile([P, P], BF16, name="swpk", tag="swpk")
                        for (dd, ss2) in [(0, 32), (32, 0), (64, 96), (96, 64)]:
                            nc.vector.tensor_copy(
                                swp[dd : dd + 32, :], pa[ss2 : ss2 + 32, :]
                            )
                        t1 = pt.tile([P, P], BF16, name="t1k", tag="t1k")
                        t2 = pt.tile([P, P], BF16, name="t2k", tag="t2k")
                        nc.vector.tensor_mul(
                            t1[:], pa[:], trig["cosk"][:, u * P : (u + 1) * P]
                        )
                        nc.vector.tensor_mul(
                            t2[:], swp[:], trig["sink"][:, u * P : (u + 1) * P]
                        )
                        nc.vector.tensor_add(kq[:, s, :], t1[:], t2[:])
                    pv = psA.tile([P, 2, SL], F32, name="pv", tag="pv")
                    for j in range(2):
                        for ks in range(KSUB):
                            nc.tensor.matmul(
                                pv[:, j, 0 : D // 2],
                                lhsT=xkv_sb[:, ks, u * P : (u + 1) * P],
                                rhs=w_sb["wv"][:, ks, j * (D // 2) : (j + 1) * (D // 2)],
                                start=(ks == 0),
                                stop=(ks == KSUB - 1),
                            )
                    for j in range(2):
                        nc.scalar.copy(
                            vloc[:, u, j * 6 : (j + 1) * 6, 0:DH],
                            pv[:, j, 0 : D // 2].rearrange("p (h d) -> p h d", d=DH),
                        )
                    nc.sync.dma_start(
                        kvin[u, 0:KQ].rearrange("(ks p n) -> p ks n", p=P, ks=KSUB),
                        kq[:],
                    )
                    nc.sync.dma_start(
                        kvin[u, KQ:].rearrange("(p h d) -> p h d", p=P, h=H),
                        vloc[:, u, :, :],
                    )
                    if fake_gather:
                        for c in range(NCORES):
                            nc.sync.dma_start(kvout[u][c], kvin[u])
                    else:
                        nc.gpsimd.collective_compute(
                            "AllGather",
                            mybir.AluOpType.bypass,
                            replica_groups=[list(range(NCORES))],
                            ins=[kvin[u].opt()],
                            outs=[kvout[u][:].opt()],
                        )
                    if u == 0:
                        # early Q subtiles: head-pair 0 can start its
                        # (ACT-bound) attention under the remaining
                        # (PE-bound) K-quarter projections
                        project_rope(
                            "wq", xq_sb, trig["cosq"], trig["sinq"], qrot_t, [0, 1]
                        )

                project_rope(
                    "wq", xq_sb, trig["cosq"], trig["sinq"], qrot_t, range(2, KSUB)
                )

            # ================= Phase B: attention =================
            if stop_after == "A":
                continue
            with (
                tc.tile_pool(name="pkv", bufs=1) as pkv,
                tc.tile_pool(name="pe", bufs=4) as pe,
                tc.tile_pool(name="pn", bufs=3) as pn,
                tc.tile_pool(name="psS", bufs=1, space="PSUM") as psS,
                tc.tile_pool(name="psO", bufs=1, space="PSUM") as psO,
            ):
                ksb_q, vsb_q = [], []
                for u in range(NQ):
                    kt = pkv.tile(
                        [P, KSUB, NCORES, P], BF16, name=f"ksbq{u}", tag=f"ksbq{u}"
                    )
                    for c in range(NCORES):
                        src = kvout[u][c, 0:KQ].rearrange(
                            "(ks p n) -> p ks n", p=P, ks=KSUB
                        )
                        # core c's cols 0:64 = global block 16u+c -> chunk slot
                        # c//2 half c%2; cols 64:128 = block 16u+8+c -> slot
                        # 4+c//2 half c%2. Chunk-contiguous keys for LDWEIGHTS.
                        po = 64 * (c % 2)
                        nc.sync.dma_start(
                            kt[:, :, c // 2, po : po + 64], src[:, :, 0:64]
                        )
                        nc.sync.dma_start(
                            kt[:, :, 4 + c // 2, po : po + 64], src[:, :, 64:128]
                        )
                    ksb_q.append(kt)
                    vt = pkv.tile(
                        [P, NCORES, H, DH + 1], BF16, name=f"vsbq{u}", tag=f"vsbq{u}"
                    )
                    for c in range(NCORES):
                        po = 64 * (c % 2)
                        nc.sync.dma_start(
                            vt[po : po + 64, c // 2, :, :],
                            kvout[u][c, KQ : KQ + 64 * VW].rearrange(
                                "(p h d) -> p h d", p=64, h=H
                            ),
                        )
                        nc.sync.dma_start(
                            vt[po : po + 64, 4 + c // 2, :, :],
                            kvout[u][c, KQ + 64 * VW :].rearrange(
                                "(p h d) -> p h d", p=64, h=H
                            ),
                        )
                    vsb_q.append(vt)

                NG3 = (NKC + 2) // 3  # 11 groups of <=3 chunks
                for hp in range(H // 2 if stop_after != "KV" else 0):
                    s = hp
                    ots = [
                        psO.tile([DH + 1, SL], F32, name=f"ot{j}", tag=f"ot{j}")
                        for j in range(2)
                    ]
                    for g in range(NG3):
                        chunks = range(3 * g, min(3 * g + 3, NKC))
                        nch = len(chunks)
                        xs = 48 * g
                        sts = [
                            psS.tile([P, 3, SL], F32, name=f"st{j}", tag=f"st{j}")
                            for j in range(2)
                        ]
                        # interleave the two heads' QK matmuls: row groups
                        # (0,*) and (64,*) run concurrently on the PE array
                        for i, kc in enumerate(chunks):
                            for j in range(2):
                                off = 64 * j
                                nc.tensor.matmul(
                                    sts[j][:, i, xs:SL],
                                    lhsT=ksb_q[kc // 8][off : off + 64, s, kc % 8, :],
                                    rhs=qrot_t[s][off : off + 64, xs:SL],
                                    start=True,
                                    stop=True,
                                )
                        expss = []
                        for j in range(2):
                            exps = pe.tile(
                                [P, 3, SL], BF16, name=f"exps{j}", tag=f"exps{j}"
                            )
                            nc.scalar.activation(
                                exps[:, 0:nch, xs:SL],
                                sts[j][:, 0:nch, xs:SL],
                                mybir.ActivationFunctionType.Exp,
                                scale=0.125,
                            )
                            mw = min(48, SL - xs)
                            nc.vector.tensor_mul(
                                exps[:, 0:nch, xs : xs + mw],
                                exps[:, 0:nch, xs : xs + mw],
                                mask_sb[:, 0:nch, 0:mw],
                            )
                            expss.append(exps)
                        for i, kc in enumerate(chunks):
                            for j in range(2):
                                nc.tensor.matmul(
                                    ots[j][:, xs:SL],
                                    lhsT=vsb_q[kc // 8][:, kc % 8, 2 * hp + j, :],
                                    rhs=expss[j][:, i, xs:SL],
                                    start=(kc == 0),
                                    stop=(kc == NKC - 1),
                                    skip_group_check=True,
                                )
                    for j in range(2):
                        h = 2 * hp + j
                        ot = ots[j]
                        # partition 64 is quadrant-aligned: the DVE can move
                        # the denominator row straight to partition 0 (probed
                        # on HW), shortening the normalize chain by a DMA hop.
                        den = pn.tile([1, SL], F32, name="den", tag="den")
                        nc.vector.tensor_copy(den[0:1, :], ot[64:65, :])
                        recip = pn.tile([1, SL], F32, name="recip", tag="recip")
                        nc.vector.reciprocal(recip[:], den[:])
                        recipb = pn.tile([64, SL], F32, name="recipb", tag="recipb")
                        nc.gpsimd.partition_broadcast(recipb[:], recip[:])
                        nc.vector.tensor_mul(osb[:, h, :], ot[0:64, :], recipb[:])

            # ================= Phase C: output projection =================
            if stop_after in ("B", "KV"):
                continue
            with (
                tc.tile_pool(name="pco", bufs=2) as pco,
                tc.tile_pool(name="psC", bufs=2, space="PSUM") as psC,
            ):
                for m in range(KSUB):
                    outp = psC.tile([P, SL], F32, name="outp", tag="outp")
                    for h in range(H):
                        nc.tensor.matmul(
                            outp[:],
                            lhsT=wo_sb[:, h, m * P : (m + 1) * P],
                            rhs=osb[:, h, :],
                            start=(h == 0),
                            stop=(h == H - 1),
                        )
                    ocp = pco.tile([P, SL], F32, name="ocp", tag="ocp")
                    nc.any.tensor_copy(ocp[:], outp[:])
                    nc.sync.dma_start(out_d.ap()[m], ocp[:])

    nc.compile()
    return nc


def _host_prep(x, position_ids, Wq, Wk, Wv, Wo):
    x2 = np.asarray(x, dtype=np.float32).reshape(S, D)
    pos = np.asarray(position_ids).reshape(S)

    fraction = (2.0 * np.arange(HALF, dtype=np.float32) / DH).astype(np.float32)
    timescale = (10000.0 ** fraction).astype(np.float32)  # [32]

    def tables(p_vec):
        sinu = (p_vec[None, :].astype(np.float32) / timescale[:, None]).astype(
            np.float32
        )
        cos = np.tile(np.cos(sinu).astype(np.float32), (4, 1))
        sin = np.sin(sinu).astype(np.float32)
        # signed for the swap formulation: first-half rows get -sin (they
        # subtract the swapped second half), second-half rows get +sin.
        sin = np.concatenate([-sin, sin, -sin, sin], axis=0)
        return cos.astype(ml_dtypes.bfloat16), sin.astype(ml_dtypes.bfloat16)

    bf = ml_dtypes.bfloat16
    weights = {
        "wq": np.ascontiguousarray(np.asarray(Wq, dtype=np.float32)).astype(bf),
        "wk": np.ascontiguousarray(np.asarray(Wk, dtype=np.float32)).astype(bf),
        "wv": np.ascontiguousarray(np.asarray(Wv, dtype=np.float32)).astype(bf),
        "wo": np.ascontiguousarray(np.asarray(Wo, dtype=np.float32)).astype(bf),
    }

    in_maps = []
    for c in range(NCORES):
        qrows = np.arange(SL) * NCORES + c
        # kv rows: 64-row blocks b with b % 8 == c, in ascending order
        kvrows = (
            (np.arange(NCORES) * NCORES + c)[:, None] * 64 + np.arange(64)[None, :]
        ).ravel()
        cosq, sinq = tables(pos[qrows])
        cosk, sink = tables(pos[kvrows])
        pp = np.arange(P)[:, None, None]
        ii = np.arange(3)[None, :, None]
        jj = np.arange(48)[None, None, :]
        mask3 = (P * ii + pp <= NCORES * jj + c).astype(ml_dtypes.bfloat16)
        m = {
            "xq": np.ascontiguousarray(x2[qrows, :].T).astype(ml_dtypes.bfloat16),
            "xkv": np.ascontiguousarray(x2[kvrows, :].T).astype(
                ml_dtypes.bfloat16
            ),
            "cosq": cosq,
            "sinq": sinq,
            "cosk": cosk,
            "sink": sink,
            "mask3": mask3,
        }
        m.update(weights)
        in_maps.append(m)
    return in_maps


def kernel(x, position_ids, Wq, Wk, Wv, Wo):
    if "nc" not in _cache:
        _cache["nc"] = _build()
    nc = _cache["nc"]
    in_maps = _host_prep(x, position_ids, Wq, Wk, Wv, Wo)
    res = bass_utils.run_bass_kernel_spmd(
        nc, in_maps, core_ids=list(range(NCORES))
    )
    out = np.empty((1, S, D), dtype=np.float32)
    for c in range(NCORES):
        outT = res.results[c]["out"].reshape(D, SL)  # [768, 512]
        out[0, c::NCORES, :] = outT.T
    return out



# revision 6
# speedup vs baseline: 1.2717x; 1.2717x over previous
"""Causal self-attention (B=1, S=4096, D=768, H=12, dh=64) on 8 TRN2 NeuronCores.

Strategy (v2 — DMA-count + engine-balance rework of the v1 baseline):
  - Sequence-parallel QKV projections + RoPE (each core projects 512 rows).
  - Queries are stride-8 interleaved (core c owns query rows c::8) so causal
    work balances and the program is SPMD-uniform; all per-core variation is
    input data (x slices, rope tables, masks).
  - KV ownership is by interleaved 128-row chunks: core c owns global chunks
    {8j+c}.  AllGather quarter u then delivers chunks 8u..8u+7 in causal
    order AND each rank's contribution is one whole chunk, so the gathered
    quarter reloads into SBUF as ONE DMA with 3KB contiguous runs.
  - K^T and V for a quarter live in one combined per-partition row of width
    1548 (768 K^T elems as (ks,n) + 12 heads x 65 V elems incl. a ones column
    that yields the softmax denominator through the AV matmul), so the
    bounce-out to DRAM is one DMA per quarter and the post-gather reload is
    one DMA per quarter.
  - Every model input loads with a single strided DMA (weights, x slices,
    packed trig tables).  Total DMA instructions ~49 vs ~245 in v1: each DMA
    costs ~650ns of serialized sequencer + HWDGE time in addition to wire
    time, so this removes ~250us of queue pressure.
  - Attention: transposed layout (keys on partitions via S^T = K^T.T @ Q^T),
    3-chunk groups with a 48-column causal offset; exp runs on the scalar
    engine (one instruction per group x head).  The causal-band mask multiply
    runs on gpsimd, off the exp->AV critical path: AV is split into a
    mask-independent main region [xs+48, SL) and a 48-wide diagonal band that
    is emitted two groups late so the Pool latency hides.
  - PE emission order per group: QK(g) first (unblocks the next exp ASAP),
    then AVmain(g-1), then AVband(g-2), keeping the tensor engine dense so
    the p-state stays at max clock.
  - Softmax normalize: DVE reciprocal of the denominator row + gpsimd
    partition-broadcast + DVE multiply, overlapped with the next head-pair.
"""

import numpy as np
import ml_dtypes

import concourse.bass as bass
import concourse.bacc as bacc
import concourse.tile as tile
import concourse.mybir as mybir
import concourse.bass_utils as bass_utils

NCORES = 8
S = 4096
D = 768
H = 12
DH = 64
HALF = 32
P = 128
SL = S // NCORES          # 512 local queries / kv rows per core
KSUB = D // P             # 6
NKC = S // P              # 32 key chunks of 128
NQ = 4                    # pipelined AllGather quarters
VW = H * (DH + 1)         # 780: V row width incl. ones col per head
CW = D + VW               # 1548: combined K^T+V per-partition row width
RQ = P * CW               # per-rank elems per quarter
NG3 = (NKC + 2) // 3      # 11 causal groups of <=3 chunks
F32 = mybir.dt.float32
BF16 = mybir.dt.bfloat16

_cache = {}


def _build(repeats=1, fake_gather=False, stop_after=None):
    nc = bacc.Bacc(
        "TRN2",
        target_bir_lowering=False,
        debug=False,
        enable_asserts=False,
        num_devices=1 if fake_gather else NCORES,
    )
    inp = {}
    for name, shape, dt in [
        ("xq", [D, SL], BF16),
        ("xkv", [D, SL], BF16),
        ("trig", [4, P, SL], BF16),   # cosq, sinq, cosk, sink
        ("mask3", [P, 3, 48], BF16),
        ("wq", [D, D], BF16),
        ("wk", [D, D], BF16),
        ("wv", [D, D], BF16),
        ("wo", [D, D], BF16),
    ]:
        inp[name] = nc.dram_tensor(name, shape, dt, kind="ExternalInput")
    out_d = nc.dram_tensor("out", [KSUB, P, SL], F32, kind="ExternalOutput")

    with tile.TileContext(nc) as tc:
      for _rep in range(repeats):
        with (
            tc.tile_pool(name="persist", bufs=1) as persist,
            tc.tile_pool(name="dram", bufs=1, space="DRAM") as dram,
        ):
            # ---- persistent tiles ----
            qrot_t = [
                persist.tile([P, SL], BF16, name=f"qrot{s_}", tag=f"qrot{s_}")
                for s_ in range(KSUB)
            ]
            osb = persist.tile([64, H, SL], BF16)
            mask_sb = persist.tile([P, 3, 48], BF16)
            trig_sb = persist.tile([P, 4, SL], BF16)
            wo_sb = persist.tile([64, H, D], BF16)
            kvt = [
                persist.tile([P, NCORES, CW], BF16, name=f"kvt{u}", tag=f"kvt{u}")
                for u in range(NQ)
            ]

            kvin = dram.tile([NQ, RQ], BF16)
            kvout = [
                dram.tile(
                    [NCORES, RQ],
                    BF16,
                    name=f"kvout{u}",
                    addr_space="Local" if fake_gather else "Shared",
                )
                for u in range(NQ)
            ]

            # ================= Phase A: projections + rope =================
            with (
                tc.tile_pool(name="pw", bufs=1) as pw,
                tc.tile_pool(name="px", bufs=1) as px,
                tc.tile_pool(name="pt", bufs=2) as pt,
                tc.tile_pool(name="psA", bufs=2, space="PSUM") as psA,
            ):
                # bulk input loads, one DMA each; K/V path inputs first so
                # quarter 0 reaches the gather as early as possible.
                w_sb = {}
                for name in ["wk", "wv", "wq"]:
                    w_sb[name] = pw.tile([P, KSUB, D], BF16, name=f"{name}_sb")
                xq_sb = px.tile([P, KSUB, SL], BF16)
                xkv_sb = px.tile([P, KSUB, SL], BF16)

                nc.sync.dma_start(
                    w_sb["wk"][:],
                    inp["wk"].ap().rearrange("(ks p) m -> p ks m", p=P),
                )
                nc.sync.dma_start(
                    xkv_sb[:], inp["xkv"].ap().rearrange("(ks p) n -> p ks n", p=P)
                )
                nc.sync.dma_start(
                    w_sb["wv"][:],
                    inp["wv"].ap().rearrange("(ks p) m -> p ks m", p=P),
                )
                nc.sync.dma_start(
                    w_sb["wq"][:],
                    inp["wq"].ap().rearrange("(ks p) m -> p ks m", p=P),
                )
                nc.sync.dma_start(
                    xq_sb[:], inp["xq"].ap().rearrange("(ks p) n -> p ks n", p=P)
                )
                nc.sync.dma_start(
                    trig_sb[:], inp["trig"].ap().rearrange("t p n -> p t n")
                )
                nc.sync.dma_start(mask_sb[:], inp["mask3"].ap())
                nc.sync.dma_start(
                    wo_sb[:], inp["wo"].ap().rearrange("(h p) e -> p h e", p=64)
                )
                cosq = trig_sb[:, 0, :]
                sinq = trig_sb[:, 1, :]
                cosk = trig_sb[:, 2, :]
                sink = trig_sb[:, 3, :]

                def project_rope_q(s):
                    paq = psA.tile([P, SL], F32, name="paq", tag="paq")
                    for ks in range(KSUB):
                        nc.tensor.matmul(
                            paq[:],
                            lhsT=w_sb["wq"][:, ks, s * P : (s + 1) * P],
                            rhs=xq_sb[:, ks, :],
                            start=(ks == 0),
                            stop=(ks == KSUB - 1),
                        )
                    pab = pt.tile([P, SL], BF16, name="pabq", tag="pabq")
                    nc.scalar.copy(pab[:], paq[:])
                    swp = pt.tile([P, SL], BF16, name="swpq", tag="swpq")
                    for (dd, ss2) in [(0, 32), (32, 0), (64, 96), (96, 64)]:
                        nc.vector.tensor_copy(
                            swp[dd : dd + 32, :], pab[ss2 : ss2 + 32, :]
                        )
                    t1 = pt.tile([P, SL], BF16, name="t1q", tag="t1q")
                    t2 = pt.tile([P, SL], BF16, name="t2q", tag="t2q")
                    nc.vector.tensor_mul(t1[:], pab[:], cosq)
                    nc.vector.tensor_mul(t2[:], swp[:], sinq)
                    nc.vector.tensor_add(qrot_t[s][:], t1[:], t2[:])

                for u in range(NQ):
                    kvs = pt.tile([P, CW], BF16, name="kvs", tag="kvs")
                    kvs_v = kvs[:, D:CW].rearrange("p (h d) -> p h d", d=DH + 1)
                    nc.vector.memset(kvs_v[:, :, DH : DH + 1], 1.0)
                    for s in range(KSUB):
                        pak = psA.tile([P, P], F32, name="pak", tag="pak")
                        for ks in range(KSUB):
                            nc.tensor.matmul(
                                pak[:],
                                lhsT=w_sb["wk"][:, ks, s * P : (s + 1) * P],
                                rhs=xkv_sb[:, ks, u * P : (u + 1) * P],
                                start=(ks == 0),
                                stop=(ks == KSUB - 1),
                            )
                        pab = pt.tile([P, P], BF16, name="pabk", tag="pabk")
                        nc.scalar.copy(pab[:], pak[:])
                        swp = pt.tile([P, P], BF16, name="swpk", tag="swpk")
                        for (dd, ss2) in [(0, 32), (32, 0), (64, 96), (96, 64)]:
                            nc.vector.tensor_copy(
                                swp[dd : dd + 32, :], pab[ss2 : ss2 + 32, :]
                            )
                        t1 = pt.tile([P, P], BF16, name="t1k", tag="t1k")
                        t2 = pt.tile([P, P], BF16, name="t2k", tag="t2k")
                        nc.vector.tensor_mul(
                            t1[:], pab[:], cosk[:, u * P : (u + 1) * P]
                        )
                        nc.vector.tensor_mul(
                            t2[:], swp[:], sink[:, u * P : (u + 1) * P]
                        )
                        nc.vector.tensor_add(kvs[:, s * P : (s + 1) * P], t1[:], t2[:])
                    # j slices are [P, SL] so each stays inside one 2KB PSUM
                    # bank (matmul outputs may not straddle banks)
                    pv = psA.tile([P, 2, SL], F32, name="pv", tag="pv")
                    for j in range(2):
                        for ks in range(KSUB):
                            nc.tensor.matmul(
                                pv[:, j, 0 : D // 2],
                                lhsT=xkv_sb[:, ks, u * P : (u + 1) * P],
                                rhs=w_sb["wv"][:, ks, j * (D // 2) : (j + 1) * (D // 2)],
                                start=(ks == 0),
                                stop=(ks == KSUB - 1),
                            )
                    for j in range(2):
                        nc.scalar.copy(
                            kvs_v[:, j * 6 : (j + 1) * 6, 0:DH],
                            pv[:, j, 0 : D // 2].rearrange("p (h d) -> p h d", d=DH),
                        )
                    nc.sync.dma_start(
                        kvin[u].rearrange("(p x) -> p x", p=P), kvs[:]
                    )
                    if fake_gather:
                        for c in range(NCORES):
                            nc.sync.dma_start(kvout[u][c], kvin[u])
                    else:
                        nc.gpsimd.collective_compute(
                            "AllGather",
                            mybir.AluOpType.bypass,
                            replica_groups=[list(range(NCORES))],
                            ins=[kvin[u].opt()],
                            outs=[kvout[u][:].opt()],
                        )
                    nc.sync.dma_start(
                        kvt[u][:],
                        kvout[u][:].rearrange("c (p x) -> p c x", p=P),
                    )
                    # early Q subtiles overlap the gather/reload pipeline
                    if u == 0:
                        project_rope_q(0)
                        project_rope_q(1)
                    elif u == 1:
                        project_rope_q(2)
                        project_rope_q(3)
                    elif u == 2:
                        project_rope_q(4)
                        project_rope_q(5)

            # ================= Phase B: attention =================
            if stop_after == "A":
                continue
            with (
                tc.tile_pool(name="pe", bufs=3) as pe,
                tc.tile_pool(name="pn", bufs=2) as pn,
                tc.tile_pool(name="psS", bufs=1, space="PSUM") as psS,
                tc.tile_pool(name="psO", bufs=1, space="PSUM") as psO,
            ):
                for hp in range(H // 2):
                    s = hp
                    ots = [
                        psO.tile([DH + 1, SL], F32, name=f"ot{j}", tag=f"ot{j}")
                        for j in range(2)
                    ]

                    # per-group AV work queue: entries (g, exps_pair)
                    stash = {}

                    # Group 0 is one full-range AV over masked exps carrying
                    # the single start=True (a start resets the whole PSUM
                    # bank, so the first-executed AV must cover all columns);
                    # later groups split into a mask-free main region and a
                    # 48-wide diagonal band, all accumulating (start=False).
                    def emit_avmain(g):
                        xs = 0 if g == 0 else 48 * g + 48
                        if xs >= SL:
                            return
                        expss = stash[g]
                        for j in range(2):
                            h = 2 * hp + j
                            for i, kc in enumerate(range(3 * g, min(3 * g + 3, NKC))):
                                nc.tensor.matmul(
                                    ots[j][:, xs:SL],
                                    lhsT=kvt[kc // 8][
                                        :, kc % 8, D + 65 * h : D + 65 * h + 65
                                    ],
                                    rhs=expss[j][:, i, xs:SL],
                                    start=(g == 0 and i == 0),
                                    stop=False,
                                    skip_group_check=True,
                                )

                    def emit_avband(g, last=False):
                        xs = 48 * g
                        mw = min(48, SL - xs)
                        expss = stash.pop(g)
                        chunks = list(range(3 * g, min(3 * g + 3, NKC)))
                        for j in range(2):
                            h = 2 * hp + j
                            for i, kc in enumerate(chunks):
                                if g == 0:
                                    continue  # covered by the full-range g=0 main
                                nc.tensor.matmul(
                                    ots[j][:, xs : xs + mw],
                                    lhsT=kvt[kc // 8][
                                        :, kc % 8, D + 65 * h : D + 65 * h + 65
                                    ],
                                    rhs=expss[j][:, i, xs : xs + mw],
                                    start=False,
                                    stop=(last and i == len(chunks) - 1),
                                    skip_group_check=True,
                                )

                    for g in range(NG3):
                        chunks = list(range(3 * g, min(3 * g + 3, NKC)))
                        nch = len(chunks)
                        xs = 48 * g
                        mw = min(48, SL - xs)
                        sts = [
                            psS.tile([P, 3, SL], F32, name=f"st{j}", tag=f"st{j}")
                            for j in range(2)
                        ]
                        for j in range(2):
                            off = 64 * j
                            for i, kc in enumerate(chunks):
                                nc.tensor.matmul(
                                    sts[j][:, i, xs:SL],
                                    lhsT=kvt[kc // 8][
                                        off : off + 64, kc % 8, s * P : (s + 1) * P
                                    ],
                                    rhs=qrot_t[s][off : off + 64, xs:SL],
                                    start=True,
                                    stop=True,
                                )
                        expss = []
                        for j in range(2):
                            exps = pe.tile(
                                [P, 3, SL], BF16, name=f"exps{j}", tag=f"exps{j}"
                            )
                            nc.scalar.activation(
                                exps[:, 0:nch, xs:SL],
                                sts[j][:, 0:nch, xs:SL],
                                mybir.ActivationFunctionType.Exp,
                                scale=0.125,
                            )
                            nc.gpsimd.tensor_mul(
                                exps[:, 0:nch, xs : xs + mw],
                                exps[:, 0:nch, xs : xs + mw],
                                mask_sb[:, 0:nch, 0:mw],
                            )
                            expss.append(exps)
                        stash[g] = expss
                        if g >= 1:
                            emit_avmain(g - 1)
                        if g >= 2:
                            emit_avband(g - 2)
                    emit_avmain(NG3 - 1)
                    emit_avband(NG3 - 2)
                    emit_avband(NG3 - 1, last=True)

                    for j in range(2):
                        h = 2 * hp + j
                        ot = ots[j]
                        den = pn.tile([1, SL], F32, name="den", tag="den")
                        nc.vector.tensor_copy(den[0:1, :], ot[64:65, :])
                        recip = pn.tile([1, SL], F32, name="recip", tag="recip")
                        nc.vector.reciprocal(recip[:], den[:])
                        recipb = pn.tile([64, SL], F32, name="recipb", tag="recipb")
                        nc.gpsimd.partition_broadcast(recipb[:], recip[:])
                        nc.vector.tensor_mul(osb[:, h, :], ot[0:64, :], recipb[:])

            # ================= Phase C: output projection =================
            if stop_after == "B":
                continue
            with (
                tc.tile_pool(name="pco", bufs=1) as pco,
                tc.tile_pool(name="psC", bufs=2, space="PSUM") as psC,
            ):
                ob_sb = pco.tile([P, KSUB, SL], F32)
                for m in range(KSUB):
                    outp = psC.tile([P, SL], F32, name="outp", tag="outp")
                    for h in range(H):
                        nc.tensor.matmul(
                            outp[:],
                            lhsT=wo_sb[:, h, m * P : (m + 1) * P],
                            rhs=osb[:, h, :],
                            start=(h == 0),
                            stop=(h == H - 1),
                        )
                    nc.scalar.copy(ob_sb[:, m, :], outp[:])
                nc.sync.dma_start(
                    out_d.ap().rearrange("m p n -> p m n"), ob_sb[:]
                )

    nc.compile()
    return nc


def _host_prep(x, position_ids, Wq, Wk, Wv, Wo):
    x2 = np.asarray(x, dtype=np.float32).reshape(S, D)
    pos = np.asarray(position_ids).reshape(S)

    fraction = (2.0 * np.arange(HALF, dtype=np.float32) / DH).astype(np.float32)
    timescale = (10000.0 ** fraction).astype(np.float32)  # [32]

    def tables(p_vec):
        sinu = (p_vec[None, :].astype(np.float32) / timescale[:, None]).astype(
            np.float32
        )
        cos = np.tile(np.cos(sinu).astype(np.float32), (4, 1))
        sin = np.sin(sinu).astype(np.float32)
        # signed for the swap formulation: first-half rows get -sin (they
        # subtract the swapped second half), second-half rows get +sin.
        sin = np.concatenate([-sin, sin, -sin, sin], axis=0)
        return cos, sin

    bf = ml_dtypes.bfloat16
    weights = {
        "wq": np.ascontiguousarray(np.asarray(Wq, dtype=np.float32)).astype(bf),
        "wk": np.ascontiguousarray(np.asarray(Wk, dtype=np.float32)).astype(bf),
        "wv": np.ascontiguousarray(np.asarray(Wv, dtype=np.float32)).astype(bf),
        "wo": np.ascontiguousarray(np.asarray(Wo, dtype=np.float32)).astype(bf),
    }

    in_maps = []
    for c in range(NCORES):
        qrows = np.arange(SL) * NCORES + c
        # kv rows: core c owns global 128-chunks {8j+c}
        kvrows = (
            (np.arange(NQ) * NCORES + c)[:, None] * P + np.arange(P)[None, :]
        ).ravel()
        cosq, sinq = tables(pos[qrows])
        cosk, sink = tables(pos[kvrows])
        trig = np.stack([cosq, sinq, cosk, sink], axis=0).astype(bf)
        pp = np.arange(P)[:, None, None]
        ii = np.arange(3)[None, :, None]
        jj = np.arange(48)[None, None, :]
        mask3 = (P * ii + pp <= NCORES * jj + c).astype(bf)
        m = {
            "xq": np.ascontiguousarray(x2[qrows, :].T).astype(bf),
            "xkv": np.ascontiguousarray(x2[kvrows, :].T).astype(bf),
            "trig": trig,
            "mask3": mask3,
        }
        m.update(weights)
        in_maps.append(m)
    return in_maps


def kernel(x, position_ids, Wq, Wk, Wv, Wo):
    if "nc" not in _cache:
        _cache["nc"] = _build()
    nc = _cache["nc"]
    in_maps = _host_prep(x, position_ids, Wq, Wk, Wv, Wo)
    res = bass_utils.run_bass_kernel_spmd(
        nc, in_maps, core_ids=list(range(NCORES))
    )
    out = np.empty((1, S, D), dtype=np.float32)
    for c in range(NCORES):
        outT = res.results[c]["out"].reshape(D, SL)  # [768, 512]
        out[0, c::NCORES, :] = outT.T
    return out


# revision 13
# speedup vs baseline: 1.3774x; 1.0832x over previous
"""Causal self-attention (B=1, S=4096, D=768, H=12, dh=64) on 8 TRN2 NeuronCores.

Strategy (v3):
  - Sequence-parallel QKV projections + RoPE (each core projects 512 rows).
  - Queries are stride-8 interleaved (core c owns query rows c::8) so causal
    work balances and the program is SPMD-uniform; all per-core variation is
    input data (x slices, rope tables, masks).
  - KV ownership is by interleaved 128-row chunks: core c owns global chunks
    {8j+c}.  AllGather quarter u then delivers chunks 8u..8u+7 in causal
    order AND each rank's contribution is one whole chunk, so the gathered
    quarter reloads into SBUF as ONE DMA per tensor with >=768B runs.
  - K^T is gathered in fp8e4m3 (halves K gather bytes; QK runs as mixed
    fp8 x bf16 matmul), V in bf16 (fp8 V fails the error budget).  V carries
    a ones column per head (memset locally after reload, not gathered) that
    yields the softmax denominator through the AV matmul.
  - Every model input loads with a single strided DMA; K/V-path inputs load
    first so quarter 0 reaches the gather ASAP, Q-path and phase-C inputs
    are deferred behind quarter-0/1 gather traffic (the DMA engine pool is
    the phase-A bottleneck).
  - Attention: transposed layout (keys on partitions via S^T = K^T.T @ Q^T),
    3-chunk groups with a 48-column causal offset; exp runs on the scalar
    engine (one instruction per group x head).  The causal-band mask multiply
    runs on gpsimd, off the exp->AV critical path: group 0 is one full-range
    AV over masked exps carrying the single start=True (a matmul start resets
    the whole PSUM bank), later groups split into a mask-independent main
    region [xs+48, SL) and a 48-wide diagonal band emitted two groups late so
    the Pool latency hides.
  - PE emission order per group: QK(g) first (unblocks the next exp ASAP),
    then AVmain(g-1), then AVband(g-2), keeping the tensor engine dense so
    the p-state stays at max clock.
  - Softmax normalize: DVE reciprocal of the denominator row + gpsimd
    partition-broadcast + DVE multiply, overlapped with the next head-pair.
  - Output projection loops h-major so only the last two heads' matmuls wait
    on the final head-pair; output staged bf16 and widened on the host.
"""

import numpy as np
import ml_dtypes

import concourse.bass as bass
import concourse.bacc as bacc
import concourse.tile as tile
import concourse.mybir as mybir
import concourse.bass_utils as bass_utils

NCORES = 8
S = 4096
D = 768
H = 12
DH = 64
HALF = 32
P = 128
SL = S // NCORES          # 512 local queries / kv rows per core
KSUB = D // P             # 6
NKC = S // P              # 32 key chunks of 128
NQ = 4                    # pipelined AllGather quarters
VW = H * (DH + 1)         # 780: V row width incl. ones col per head
RQK = P * D               # per-rank K elems per quarter
RQV = P * VW              # per-rank V elems per quarter (incl. ones col)
NG3 = (NKC + 2) // 3      # 11 causal groups of <=3 chunks
F32 = mybir.dt.float32
BF16 = mybir.dt.bfloat16
F8 = mybir.dt.float8e4

_cache = {}


def _build(repeats=1, fake_gather=False, stop_after=None):
    nc = bacc.Bacc(
        "TRN2",
        target_bir_lowering=False,
        debug=False,
        enable_asserts=False,
        num_devices=1 if fake_gather else NCORES,
    )
    inp = {}
    for name, shape, dt in [
        ("xq", [D, SL], BF16),
        ("xkv", [D, SL], BF16),
        ("trig", [4, P, SL], BF16),   # cosq, sinq, cosk, sink
        ("mask3", [P, 3, 48], BF16),
        ("wq", [D, D], BF16),
        ("wk", [D, D], BF16),
        ("wv", [D, D], BF16),
        ("wo", [D, D], BF16),
    ]:
        inp[name] = nc.dram_tensor(name, shape, dt, kind="ExternalInput")
    out_d = nc.dram_tensor("out", [KSUB, P, SL], BF16, kind="ExternalOutput")

    with tile.TileContext(nc) as tc:
      for _rep in range(repeats):
        with (
            tc.tile_pool(name="persist", bufs=1) as persist,
            tc.tile_pool(name="dram", bufs=1, space="DRAM") as dram,
        ):
            # ---- persistent tiles ----
            qrot_t = [
                persist.tile([P, SL], BF16, name=f"qrot{s_}", tag=f"qrot{s_}")
                for s_ in range(KSUB)
            ]
            osb = persist.tile([64, H, SL], BF16)
            mask_sb = persist.tile([P, 3, 48], BF16)
            trig_sb = persist.tile([P, 4, SL], BF16)
            wo_sb = persist.tile([64, H, D], BF16)
            kvtK = [
                persist.tile([P, NCORES, D], F8, name=f"kvtK{u}", tag=f"kvtK{u}")
                for u in range(NQ)
            ]
            kvtV = [
                persist.tile(
                    [P, NCORES, H, DH + 1], BF16, name=f"kvtV{u}", tag=f"kvtV{u}"
                )
                for u in range(NQ)
            ]

            kvinK = dram.tile([NQ, RQK], F8)
            kvinV = dram.tile([NQ, RQV], BF16)
            kvoutK = [
                dram.tile(
                    [NCORES, RQK],
                    F8,
                    name=f"kvoutK{u}",
                    addr_space="Local" if fake_gather else "Shared",
                )
                for u in range(NQ)
            ]
            kvoutV = [
                dram.tile(
                    [NCORES, RQV],
                    BF16,
                    name=f"kvoutV{u}",
                    addr_space="Local" if fake_gather else "Shared",
                )
                for u in range(NQ)
            ]

            # ================= Phase A: projections + rope =================
            with (
                tc.tile_pool(name="pw", bufs=1) as pw,
                tc.tile_pool(name="px", bufs=1) as px,
                tc.tile_pool(name="pt", bufs=2) as pt,
                tc.tile_pool(name="psA", bufs=2, space="PSUM") as psA,
            ):
                w_sb = {}
                for name in ["wk", "wv", "wq"]:
                    w_sb[name] = pw.tile([P, KSUB, D], BF16, name=f"{name}_sb")
                xq_sb = px.tile([P, KSUB, SL], BF16)
                xkv_sb = px.tile([P, KSUB, SL], BF16)

                # K/V-path inputs first: the DMA engine pool is the phase-A
                # bottleneck and quarter 0 gates the attention start.
                nc.sync.dma_start(
                    w_sb["wk"][:],
                    inp["wk"].ap().rearrange("(ks p) m -> p ks m", p=P),
                )
                nc.sync.dma_start(
                    xkv_sb[:], inp["xkv"].ap().rearrange("(ks p) n -> p ks n", p=P)
                )
                nc.sync.dma_start(
                    trig_sb[:], inp["trig"].ap().rearrange("t p n -> p t n")
                )
                nc.sync.dma_start(
                    w_sb["wv"][:],
                    inp["wv"].ap().rearrange("(ks p) m -> p ks m", p=P),
                )
                cosq = trig_sb[:, 0, :]
                sinq = trig_sb[:, 1, :]
                cosk = trig_sb[:, 2, :]
                sink = trig_sb[:, 3, :]

                def load_q_inputs():
                    nc.sync.dma_start(
                        w_sb["wq"][:],
                        inp["wq"].ap().rearrange("(ks p) m -> p ks m", p=P),
                    )
                    nc.sync.dma_start(
                        xq_sb[:], inp["xq"].ap().rearrange("(ks p) n -> p ks n", p=P)
                    )

                def load_late_inputs():
                    nc.sync.dma_start(mask_sb[:], inp["mask3"].ap())
                    nc.sync.dma_start(
                        wo_sb[:], inp["wo"].ap().rearrange("(h p) e -> p h e", p=64)
                    )

                def project_rope_q(s):
                    paq = psA.tile([P, SL], F32, name="paq", tag="paq")
                    for ks in range(KSUB):
                        nc.tensor.matmul(
                            paq[:],
                            lhsT=w_sb["wq"][:, ks, s * P : (s + 1) * P],
                            rhs=xq_sb[:, ks, :],
                            start=(ks == 0),
                            stop=(ks == KSUB - 1),
                        )
                    pab = pt.tile([P, SL], BF16, name="pabq", tag="pabq")
                    nc.scalar.copy(pab[:], paq[:])
                    swp = pt.tile([P, SL], BF16, name="swpq", tag="swpq")
                    for (dd, ss2) in [(0, 32), (32, 0), (64, 96), (96, 64)]:
                        nc.vector.tensor_copy(
                            swp[dd : dd + 32, :], pab[ss2 : ss2 + 32, :]
                        )
                    t1 = pt.tile([P, SL], BF16, name="t1q", tag="t1q")
                    t2 = pt.tile([P, SL], BF16, name="t2q", tag="t2q")
                    nc.vector.tensor_mul(t1[:], pab[:], cosq)
                    nc.vector.tensor_mul(t2[:], swp[:], sinq)
                    nc.vector.tensor_add(qrot_t[s][:], t1[:], t2[:])

                for u in range(NQ):
                    kvsK = pt.tile([P, D], F8, name="kvsK", tag="kvsK")
                    kvsV = pt.tile([P, H, DH + 1], BF16, name="kvsV", tag="kvsV")
                    nc.vector.memset(kvsV[:, :, DH : DH + 1], 1.0)
                    for s in range(KSUB):
                        pak = psA.tile([P, P], F32, name="pak", tag="pak")
                        for ks in range(KSUB):
                            nc.tensor.matmul(
                                pak[:],
                                lhsT=w_sb["wk"][:, ks, s * P : (s + 1) * P],
                                rhs=xkv_sb[:, ks, u * P : (u + 1) * P],
                                start=(ks == 0),
                                stop=(ks == KSUB - 1),
                            )
                        pab = pt.tile([P, P], BF16, name="pabk", tag="pabk")
                        nc.scalar.copy(pab[:], pak[:])
                        swp = pt.tile([P, P], BF16, name="swpk", tag="swpk")
                        for (dd, ss2) in [(0, 32), (32, 0), (64, 96), (96, 64)]:
                            nc.vector.tensor_copy(
                                swp[dd : dd + 32, :], pab[ss2 : ss2 + 32, :]
                            )
                        t1 = pt.tile([P, P], BF16, name="t1k", tag="t1k")
                        t2 = pt.tile([P, P], BF16, name="t2k", tag="t2k")
                        nc.vector.tensor_mul(
                            t1[:], pab[:], cosk[:, u * P : (u + 1) * P]
                        )
                        nc.vector.tensor_mul(
                            t2[:], swp[:], sink[:, u * P : (u + 1) * P]
                        )
                        nc.vector.tensor_add(
                            kvsK[:, s * P : (s + 1) * P], t1[:], t2[:]
                        )
                    # j slices are [P, SL] so each stays inside one 2KB PSUM
                    # bank (matmul outputs may not straddle banks)
                    pv = psA.tile([P, 2, SL], F32, name="pv", tag="pv")
                    for j in range(2):
                        for ks in range(KSUB):
                            nc.tensor.matmul(
                                pv[:, j, 0 : D // 2],
                                lhsT=xkv_sb[:, ks, u * P : (u + 1) * P],
                                rhs=w_sb["wv"][:, ks, j * (D // 2) : (j + 1) * (D // 2)],
                                start=(ks == 0),
                                stop=(ks == KSUB - 1),
                            )
                    for j in range(2):
                        nc.scalar.copy(
                            kvsV[:, j * 6 : (j + 1) * 6, 0:DH],
                            pv[:, j, 0 : D // 2].rearrange("p (h d) -> p h d", d=DH),
                        )
                    nc.sync.dma_start(
                        kvinK[u].rearrange("(p x) -> p x", p=P), kvsK[:]
                    )
                    nc.sync.dma_start(
                        kvinV[u].rearrange("(p h d) -> p h d", p=P, h=H), kvsV[:]
                    )
                    if fake_gather:
                        for c in range(NCORES):
                            nc.sync.dma_start(kvoutK[u][c], kvinK[u])
                            nc.sync.dma_start(kvoutV[u][c], kvinV[u])
                    else:
                        nc.gpsimd.collective_compute(
                            "AllGather",
                            mybir.AluOpType.bypass,
                            replica_groups=[list(range(NCORES))],
                            ins=[kvinK[u].opt()],
                            outs=[kvoutK[u][:].opt()],
                        )
                        nc.gpsimd.collective_compute(
                            "AllGather",
                            mybir.AluOpType.bypass,
                            replica_groups=[list(range(NCORES))],
                            ins=[kvinV[u].opt()],
                            outs=[kvoutV[u][:].opt()],
                        )
                    nc.sync.dma_start(
                        kvtK[u][:],
                        kvoutK[u][:].rearrange("c (p x) -> p c x", p=P),
                    )
                    nc.sync.dma_start(
                        kvtV[u][:],
                        kvoutV[u][:].rearrange("c (p h d) -> p c h d", p=P, h=H),
                    )
                    if u == 0:
                        load_q_inputs()
                        project_rope_q(0)
                        project_rope_q(1)
                    elif u == 1:
                        load_late_inputs()
                        project_rope_q(2)
                        project_rope_q(3)
                    elif u == 2:
                        project_rope_q(4)
                        project_rope_q(5)

            # ================= Phase B: attention =================
            if stop_after == "A":
                continue
            with (
                tc.tile_pool(name="pe", bufs=3) as pe,
                tc.tile_pool(name="pn", bufs=2) as pn,
                tc.tile_pool(name="psS", bufs=1, space="PSUM") as psS,
                tc.tile_pool(name="psO", bufs=1, space="PSUM") as psO,
            ):
                for hp in range(H // 2):
                    s = hp
                    ots = [
                        psO.tile([DH + 1, SL], F32, name=f"ot{j}", tag=f"ot{j}")
                        for j in range(2)
                    ]

                    stash = {}

                    # Group 0 is one full-range AV over masked exps carrying
                    # the single start=True (a start resets the whole PSUM
                    # bank, so the first-executed AV must cover all columns);
                    # later groups split into a mask-free main region and a
                    # 48-wide diagonal band, all accumulating (start=False).
                    def emit_avmain(g):
                        xs = 0 if g == 0 else 48 * g + 48
                        if xs >= SL:
                            return
                        expss = stash[g]
                        for j in range(2):
                            h = 2 * hp + j
                            for i, kc in enumerate(range(3 * g, min(3 * g + 3, NKC))):
                                nc.tensor.matmul(
                                    ots[j][:, xs:SL],
                                    lhsT=kvtV[kc // 8][:, kc % 8, h, :],
                                    rhs=expss[j][:, i, xs:SL],
                                    start=(g == 0 and i == 0),
                                    stop=False,
                                    skip_group_check=True,
                                )

                    def emit_avband(g, last=False):
                        xs = 48 * g
                        mw = min(48, SL - xs)
                        expss = stash.pop(g)
                        chunks = list(range(3 * g, min(3 * g + 3, NKC)))
                        for j in range(2):
                            h = 2 * hp + j
                            for i, kc in enumerate(chunks):
                                if g == 0:
                                    continue  # covered by the full-range g=0 main
                                nc.tensor.matmul(
                                    ots[j][:, xs : xs + mw],
                                    lhsT=kvtV[kc // 8][:, kc % 8, h, :],
                                    rhs=expss[j][:, i, xs : xs + mw],
                                    start=False,
                                    stop=(last and i == len(chunks) - 1),
                                    skip_group_check=True,
                                )

                    for g in range(NG3):
                        chunks = list(range(3 * g, min(3 * g + 3, NKC)))
                        nch = len(chunks)
                        xs = 48 * g
                        mw = min(48, SL - xs)
                        sts = [
                            psS.tile([P, 3, SL], F32, name=f"st{j}", tag=f"st{j}")
                            for j in range(2)
                        ]
                        for j in range(2):
                            off = 64 * j
                            for i, kc in enumerate(chunks):
                                nc.tensor.matmul(
                                    sts[j][:, i, xs:SL],
                                    lhsT=kvtK[kc // 8][
                                        off : off + 64, kc % 8, s * P : (s + 1) * P
                                    ],
                                    rhs=qrot_t[s][off : off + 64, xs:SL],
                                    start=True,
                                    stop=True,
                                )
                        expss = []
                        for j in range(2):
                            exps = pe.tile(
                                [P, 3, SL], BF16, name=f"exps{j}", tag=f"exps{j}"
                            )
                            nc.scalar.activation(
                                exps[:, 0:nch, xs:SL],
                                sts[j][:, 0:nch, xs:SL],
                                mybir.ActivationFunctionType.Exp,
                                scale=0.125,
                            )
                            nc.gpsimd.tensor_mul(
                                exps[:, 0:nch, xs : xs + mw],
                                exps[:, 0:nch, xs : xs + mw],
                                mask_sb[:, 0:nch, 0:mw],
                            )
                            expss.append(exps)
                        stash[g] = expss
                        if g >= 1:
                            emit_avmain(g - 1)
                        if g >= 2:
                            emit_avband(g - 2)
                    emit_avmain(NG3 - 1)
                    emit_avband(NG3 - 2)
                    emit_avband(NG3 - 1, last=True)

                    for j in range(2):
                        h = 2 * hp + j
                        ot = ots[j]
                        den = pn.tile([1, SL], F32, name="den", tag="den")
                        nc.vector.tensor_copy(den[0:1, :], ot[64:65, :])
                        recip = pn.tile([1, SL], F32, name="recip", tag="recip")
                        nc.vector.reciprocal(recip[:], den[:])
                        recipb = pn.tile([64, SL], F32, name="recipb", tag="recipb")
                        nc.gpsimd.partition_broadcast(recipb[:], recip[:])
                        nc.vector.tensor_mul(osb[:, h, :], ot[0:64, :], recipb[:])

            # ================= Phase C: output projection =================
            if stop_after == "B":
                continue
            with (
                tc.tile_pool(name="pco", bufs=1) as pco,
                tc.tile_pool(name="psC", bufs=1, space="PSUM") as psC,
            ):
                ob_sb = pco.tile([P, KSUB, SL], BF16)
                outps = [
                    psC.tile([P, SL], F32, name=f"outp{m}", tag=f"outp{m}")
                    for m in range(KSUB)
                ]
                # h-major: only the last head-pair's matmuls wait on hp5
                for h in range(H):
                    for m in range(KSUB):
                        nc.tensor.matmul(
                            outps[m][:],
                            lhsT=wo_sb[:, h, m * P : (m + 1) * P],
                            rhs=osb[:, h, :],
                            start=(h == 0),
                            stop=(h == H - 1),
                        )
                for m in range(KSUB):
                    nc.vector.tensor_copy(ob_sb[:, m, :], outps[m][:])
                nc.sync.dma_start(
                    out_d.ap().rearrange("m p n -> p m n"), ob_sb[:]
                )

    nc.compile()
    return nc


def _host_prep(x, position_ids, Wq, Wk, Wv, Wo):
    x2 = np.asarray(x, dtype=np.float32).reshape(S, D)
    pos = np.asarray(position_ids).reshape(S)

    fraction = (2.0 * np.arange(HALF, dtype=np.float32) / DH).astype(np.float32)
    timescale = (10000.0 ** fraction).astype(np.float32)  # [32]

    def tables(p_vec):
        sinu = (p_vec[None, :].astype(np.float32) / timescale[:, None]).astype(
            np.float32
        )
        cos = np.tile(np.cos(sinu).astype(np.float32), (4, 1))
        sin = np.sin(sinu).astype(np.float32)
        # signed for the swap formulation: first-half rows get -sin (they
        # subtract the swapped second half), second-half rows get +sin.
        sin = np.concatenate([-sin, sin, -sin, sin], axis=0)
        return cos, sin

    bf = ml_dtypes.bfloat16
    weights = {
        "wq": np.ascontiguousarray(np.asarray(Wq, dtype=np.float32)).astype(bf),
        "wk": np.ascontiguousarray(np.asarray(Wk, dtype=np.float32)).astype(bf),
        "wv": np.ascontiguousarray(np.asarray(Wv, dtype=np.float32)).astype(bf),
        "wo": np.ascontiguousarray(np.asarray(Wo, dtype=np.float32)).astype(bf),
    }

    in_maps = []
    for c in range(NCORES):
        qrows = np.arange(SL) * NCORES + c
        # kv rows: core c owns global 128-chunks {8j+c}
        kvrows = (
            (np.arange(NQ) * NCORES + c)[:, None] * P + np.arange(P)[None, :]
        ).ravel()
        cosq, sinq = tables(pos[qrows])
        cosk, sink = tables(pos[kvrows])
        trig = np.stack([cosq, sinq, cosk, sink], axis=0).astype(bf)
        pp = np.arange(P)[:, None, None]
        ii = np.arange(3)[None, :, None]
        jj = np.arange(48)[None, None, :]
        mask3 = (P * ii + pp <= NCORES * jj + c).astype(bf)
        m = {
            "xq": np.ascontiguousarray(x2[qrows, :].T).astype(bf),
            "xkv": np.ascontiguousarray(x2[kvrows, :].T).astype(bf),
            "trig": trig,
            "mask3": mask3,
        }
        m.update(weights)
        in_maps.append(m)
    return in_maps


def kernel(x, position_ids, Wq, Wk, Wv, Wo):
    if "nc" not in _cache:
        _cache["nc"] = _build()
    nc = _cache["nc"]
    in_maps = _host_prep(x, position_ids, Wq, Wk, Wv, Wo)
    res = bass_utils.run_bass_kernel_spmd(
        nc, in_maps, core_ids=list(range(NCORES))
    )
    out = np.empty((1, S, D), dtype=np.float32)
    for c in range(NCORES):
        outT = res.results[c]["out"].astype(np.float32).reshape(D, SL)
        out[0, c::NCORES, :] = outT.T
    return out


# revision 14
# speedup vs baseline: 1.3961x; 1.0135x over previous
"""Causal self-attention (B=1, S=4096, D=768, H=12, dh=64) on 8 TRN2 NeuronCores.

Strategy (v3):
  - Sequence-parallel QKV projections + RoPE (each core projects 512 rows).
  - Queries are stride-8 interleaved (core c owns query rows c::8) so causal
    work balances and the program is SPMD-uniform; all per-core variation is
    input data (x slices, rope tables, masks).
  - KV ownership is by interleaved 128-row chunks: core c owns global chunks
    {8j+c}.  AllGather quarter u then delivers chunks 8u..8u+7 in causal
    order AND each rank's contribution is one whole chunk, so the gathered
    quarter reloads into SBUF as ONE DMA per tensor with >=768B runs.
  - K^T is gathered in fp8e4m3 (halves K gather bytes; QK runs as mixed
    fp8 x bf16 matmul), V in bf16 (fp8 V fails the error budget).  V carries
    a ones column per head (memset locally after reload, not gathered) that
    yields the softmax denominator through the AV matmul.
  - Every model input loads with a single strided DMA; K/V-path inputs load
    first so quarter 0 reaches the gather ASAP, Q-path and phase-C inputs
    are deferred behind quarter-0/1 gather traffic (the DMA engine pool is
    the phase-A bottleneck).
  - Attention: transposed layout (keys on partitions via S^T = K^T.T @ Q^T),
    3-chunk groups with a 48-column causal offset; exp runs on the scalar
    engine (one instruction per group x head).  The causal-band mask multiply
    runs on gpsimd, off the exp->AV critical path: group 0 is one full-range
    AV over masked exps carrying the single start=True (a matmul start resets
    the whole PSUM bank), later groups split into a mask-independent main
    region [xs+48, SL) and a 48-wide diagonal band emitted two groups late so
    the Pool latency hides.
  - PE emission order per group: QK(g) first (unblocks the next exp ASAP),
    then AVmain(g-1), then AVband(g-2), keeping the tensor engine dense so
    the p-state stays at max clock.
  - Softmax normalize: DVE reciprocal of the denominator row + gpsimd
    partition-broadcast + DVE multiply, overlapped with the next head-pair.
  - Output projection loops h-major so only the last two heads' matmuls wait
    on the final head-pair; output staged bf16 and widened on the host.
"""

import numpy as np
import ml_dtypes

import concourse.bass as bass
import concourse.bacc as bacc
import concourse.tile as tile
import concourse.mybir as mybir
import concourse.bass_utils as bass_utils

NCORES = 8
S = 4096
D = 768
H = 12
DH = 64
HALF = 32
P = 128
SL = S // NCORES          # 512 local queries / kv rows per core
KSUB = D // P             # 6
NKC = S // P              # 32 key chunks of 128
NQ = 4                    # pipelined AllGather quarters
VW = H * (DH + 1)         # 780: V row width incl. ones col per head
RQK = P * D               # per-rank K elems per quarter
RQV = P * VW              # per-rank V elems per quarter (incl. ones col)
NG3 = (NKC + 2) // 3      # 11 causal groups of <=3 chunks
F32 = mybir.dt.float32
BF16 = mybir.dt.bfloat16
F8 = mybir.dt.float8e4

_cache = {}


def _build(repeats=1, fake_gather=False, stop_after=None):
    nc = bacc.Bacc(
        "TRN2",
        target_bir_lowering=False,
        debug=False,
        enable_asserts=False,
        num_devices=1 if fake_gather else NCORES,
    )
    inp = {}
    for name, shape, dt in [
        ("xq", [D, SL], BF16),
        ("xkv", [D, SL], BF16),
        ("trig", [4, P, SL], BF16),   # cosq, sinq, cosk, sink
        ("mask3", [P, 3, 48], BF16),
        ("wq", [D, D], BF16),
        ("wk", [D, D], BF16),
        ("wv", [D, D], BF16),
        ("wo", [D, D], BF16),
    ]:
        inp[name] = nc.dram_tensor(name, shape, dt, kind="ExternalInput")
    out_d = nc.dram_tensor("out", [KSUB, P, SL], BF16, kind="ExternalOutput")

    with tile.TileContext(nc) as tc:
      for _rep in range(repeats):
        with (
            tc.tile_pool(name="persist", bufs=1) as persist,
            tc.tile_pool(name="dram", bufs=1, space="DRAM") as dram,
        ):
            # ---- persistent tiles ----
            qrot_t = [
                persist.tile([P, SL], BF16, name=f"qrot{s_}", tag=f"qrot{s_}")
                for s_ in range(KSUB)
            ]
            osb = persist.tile([64, H, SL], BF16)
            mask_sb = persist.tile([P, 3, 48], BF16)
            trig_sb = persist.tile([P, 4, SL], BF16)
            wo_sb = persist.tile([64, H, D], BF16)
            kvtK = [
                persist.tile([P, NCORES, D], F8, name=f"kvtK{u}", tag=f"kvtK{u}")
                for u in range(NQ)
            ]
            kvtV = [
                persist.tile(
                    [P, NCORES, H, DH + 1], BF16, name=f"kvtV{u}", tag=f"kvtV{u}"
                )
                for u in range(NQ)
            ]

            kvinK = dram.tile([NQ, RQK], F8)
            kvinV = dram.tile([NQ, RQV], BF16)
            kvoutK = [
                dram.tile(
                    [NCORES, RQK],
                    F8,
                    name=f"kvoutK{u}",
                    addr_space="Local" if fake_gather else "Shared",
                )
                for u in range(NQ)
            ]
            kvoutV = [
                dram.tile(
                    [NCORES, RQV],
                    BF16,
                    name=f"kvoutV{u}",
                    addr_space="Local" if fake_gather else "Shared",
                )
                for u in range(NQ)
            ]

            # ================= Phase A: projections + rope =================
            with (
                tc.tile_pool(name="pw", bufs=1) as pw,
                tc.tile_pool(name="px", bufs=1) as px,
                tc.tile_pool(name="pt", bufs=2) as pt,
                tc.tile_pool(name="psA", bufs=2, space="PSUM") as psA,
            ):
                w_sb = {}
                for name in ["wk", "wv", "wq"]:
                    w_sb[name] = pw.tile([P, KSUB, D], BF16, name=f"{name}_sb")
                xq_sb = px.tile([P, KSUB, SL], BF16)
                xkv_sb = px.tile([P, KSUB, SL], BF16)

                # K/V-path inputs first: the DMA engine pool is the phase-A
                # bottleneck and quarter 0 gates the attention start.
                nc.sync.dma_start(
                    w_sb["wk"][:],
                    inp["wk"].ap().rearrange("(ks p) m -> p ks m", p=P),
                )
                nc.sync.dma_start(
                    xkv_sb[:], inp["xkv"].ap().rearrange("(ks p) n -> p ks n", p=P)
                )
                nc.sync.dma_start(
                    trig_sb[:], inp["trig"].ap().rearrange("t p n -> p t n")
                )
                nc.sync.dma_start(
                    w_sb["wv"][:],
                    inp["wv"].ap().rearrange("(ks p) m -> p ks m", p=P),
                )
                cosq = trig_sb[:, 0, :]
                sinq = trig_sb[:, 1, :]
                cosk = trig_sb[:, 2, :]
                sink = trig_sb[:, 3, :]

                def load_q_inputs():
                    nc.sync.dma_start(
                        w_sb["wq"][:],
                        inp["wq"].ap().rearrange("(ks p) m -> p ks m", p=P),
                    )
                    nc.sync.dma_start(
                        xq_sb[:], inp["xq"].ap().rearrange("(ks p) n -> p ks n", p=P)
                    )

                def load_late_inputs():
                    nc.sync.dma_start(mask_sb[:], inp["mask3"].ap())
                    nc.sync.dma_start(
                        wo_sb[:], inp["wo"].ap().rearrange("(h p) e -> p h e", p=64)
                    )

                def project_rope_q(s):
                    paq = psA.tile([P, SL], F32, name="paq", tag="paq")
                    for ks in range(KSUB):
                        nc.tensor.matmul(
                            paq[:],
                            lhsT=w_sb["wq"][:, ks, s * P : (s + 1) * P],
                            rhs=xq_sb[:, ks, :],
                            start=(ks == 0),
                            stop=(ks == KSUB - 1),
                        )
                    pab = pt.tile([P, SL], BF16, name="pabq", tag="pabq")
                    nc.scalar.copy(pab[:], paq[:])
                    swp = pt.tile([P, SL], BF16, name="swpq", tag="swpq")
                    for (dd, ss2) in [(0, 32), (32, 0), (64, 96), (96, 64)]:
                        nc.vector.tensor_copy(
                            swp[dd : dd + 32, :], pab[ss2 : ss2 + 32, :]
                        )
                    t1 = pt.tile([P, SL], BF16, name="t1q", tag="t1q")
                    t2 = pt.tile([P, SL], BF16, name="t2q", tag="t2q")
                    nc.vector.tensor_mul(t1[:], pab[:], cosq)
                    nc.vector.tensor_mul(t2[:], swp[:], sinq)
                    nc.vector.tensor_add(qrot_t[s][:], t1[:], t2[:])

                for u in range(NQ):
                    kvsK = pt.tile([P, D], F8, name="kvsK", tag="kvsK")
                    kvsV = pt.tile([P, H, DH + 1], BF16, name="kvsV", tag="kvsV")
                    nc.vector.memset(kvsV[:, :, DH : DH + 1], 1.0)
                    for s in range(KSUB):
                        pak = psA.tile([P, P], F32, name="pak", tag="pak")
                        for ks in range(KSUB):
                            nc.tensor.matmul(
                                pak[:],
                                lhsT=w_sb["wk"][:, ks, s * P : (s + 1) * P],
                                rhs=xkv_sb[:, ks, u * P : (u + 1) * P],
                                start=(ks == 0),
                                stop=(ks == KSUB - 1),
                            )
                        pab = pt.tile([P, P], BF16, name="pabk", tag="pabk")
                        nc.scalar.copy(pab[:], pak[:])
                        swp = pt.tile([P, P], BF16, name="swpk", tag="swpk")
                        for (dd, ss2) in [(0, 32), (32, 0), (64, 96), (96, 64)]:
                            nc.vector.tensor_copy(
                                swp[dd : dd + 32, :], pab[ss2 : ss2 + 32, :]
                            )
                        t1 = pt.tile([P, P], BF16, name="t1k", tag="t1k")
                        t2 = pt.tile([P, P], BF16, name="t2k", tag="t2k")
                        nc.vector.tensor_mul(
                            t1[:], pab[:], cosk[:, u * P : (u + 1) * P]
                        )
                        nc.vector.tensor_mul(
                            t2[:], swp[:], sink[:, u * P : (u + 1) * P]
                        )
                        nc.vector.tensor_add(
                            kvsK[:, s * P : (s + 1) * P], t1[:], t2[:]
                        )
                    # j slices are [P, SL] so each stays inside one 2KB PSUM
                    # bank (matmul outputs may not straddle banks)
                    pv = psA.tile([P, 2, SL], F32, name="pv", tag="pv")
                    for j in range(2):
                        for ks in range(KSUB):
                            nc.tensor.matmul(
                                pv[:, j, 0 : D // 2],
                                lhsT=xkv_sb[:, ks, u * P : (u + 1) * P],
                                rhs=w_sb["wv"][:, ks, j * (D // 2) : (j + 1) * (D // 2)],
                                start=(ks == 0),
                                stop=(ks == KSUB - 1),
                            )
                    for j in range(2):
                        nc.scalar.copy(
                            kvsV[:, j * 6 : (j + 1) * 6, 0:DH],
                            pv[:, j, 0 : D // 2].rearrange("p (h d) -> p h d", d=DH),
                        )
                    nc.sync.dma_start(
                        kvinK[u].rearrange("(p x) -> p x", p=P), kvsK[:]
                    )
                    nc.sync.dma_start(
                        kvinV[u].rearrange("(p h d) -> p h d", p=P, h=H), kvsV[:]
                    )
                    if fake_gather:
                        for c in range(NCORES):
                            nc.sync.dma_start(kvoutK[u][c], kvinK[u])
                            nc.sync.dma_start(kvoutV[u][c], kvinV[u])
                    else:
                        nc.gpsimd.collective_compute(
                            "AllGather",
                            mybir.AluOpType.bypass,
                            replica_groups=[list(range(NCORES))],
                            ins=[kvinK[u].opt()],
                            outs=[kvoutK[u][:].opt()],
                        )
                        nc.gpsimd.collective_compute(
                            "AllGather",
                            mybir.AluOpType.bypass,
                            replica_groups=[list(range(NCORES))],
                            ins=[kvinV[u].opt()],
                            outs=[kvoutV[u][:].opt()],
                        )
                    nc.sync.dma_start(
                        kvtK[u][:],
                        kvoutK[u][:].rearrange("c (p x) -> p c x", p=P),
                    )
                    nc.sync.dma_start(
                        kvtV[u][:],
                        kvoutV[u][:].rearrange("c (p h d) -> p c h d", p=P, h=H),
                    )
                    if u == 0:
                        load_q_inputs()
                        project_rope_q(0)
                        project_rope_q(1)
                    elif u == 1:
                        load_late_inputs()
                        project_rope_q(2)
                        project_rope_q(3)
                    elif u == 2:
                        project_rope_q(4)
                        project_rope_q(5)

            # ================= Phase B: attention =================
            # Quarter-aligned waves: wave w covers causal groups whose chunks
            # live in quarters <= w, iterating ALL head-pairs per wave, so
            # attention consumption matches the gather pipeline's delivery
            # rate instead of head-pair 0 burning through every quarter and
            # stalling.  Per-(hp,wave) AV partials accumulate in PSUM and are
            # flushed into an SBUF accumulator by DVE adds.
            if stop_after == "A":
                continue
            WAVES = [[0, 1], [2, 3, 4], [5, 6, 7], [8, 9, 10]]
            with (
                tc.tile_pool(name="pe", bufs=3) as pe,
                tc.tile_pool(name="pn", bufs=2) as pn,
                tc.tile_pool(name="pacc", bufs=1) as pacc,
                tc.tile_pool(name="psS", bufs=1, space="PSUM") as psS,
                tc.tile_pool(name="psO", bufs=1, space="PSUM") as psO,
            ):
                acc = pacc.tile([DH + 1, H, SL], F32)
                for w, groups in enumerate(WAVES):
                  g_min = groups[0]
                  xsw = 48 * g_min
                  for hp in range(H // 2):
                    s = hp
                    ots = [
                        psO.tile([DH + 1, SL], F32, name=f"ot{j}", tag=f"ot{j}")
                        for j in range(2)
                    ]

                    stash = {}

                    # The wave's first group emits one full-range AV over
                    # masked exps carrying the single start=True (a matmul
                    # start resets the whole PSUM bank, so the first-executed
                    # AV must cover the wave's whole column range); later
                    # groups split into a mask-free main region and a 48-wide
                    # diagonal band emitted late so the Pool mask hides.
                    def emit_avmain(g):
                        xs = xsw if g == g_min else 48 * g + 48
                        if xs >= SL:
                            return
                        expss = stash[g]
                        for j in range(2):
                            h = 2 * hp + j
                            for i, kc in enumerate(range(3 * g, min(3 * g + 3, NKC))):
                                nc.tensor.matmul(
                                    ots[j][:, xs:SL],
                                    lhsT=kvtV[kc // 8][:, kc % 8, h, :],
                                    rhs=expss[j][:, i, xs:SL],
                                    start=(g == g_min and i == 0),
                                    stop=False,
                                    skip_group_check=True,
                                )

                    def emit_avband(g, last=False):
                        xs = 48 * g
                        mw = min(48, SL - xs)
                        expss = stash.pop(g)
                        chunks = list(range(3 * g, min(3 * g + 3, NKC)))
                        for j in range(2):
                            h = 2 * hp + j
                            for i, kc in enumerate(chunks):
                                if g == g_min:
                                    continue  # covered by the full-range main
                                nc.tensor.matmul(
                                    ots[j][:, xs : xs + mw],
                                    lhsT=kvtV[kc // 8][:, kc % 8, h, :],
                                    rhs=expss[j][:, i, xs : xs + mw],
                                    start=False,
                                    stop=(last and i == len(chunks) - 1),
                                    skip_group_check=True,
                                )

                    for gi, g in enumerate(groups):
                        chunks = list(range(3 * g, min(3 * g + 3, NKC)))
                        nch = len(chunks)
                        xs = 48 * g
                        mw = min(48, SL - xs)
                        sts = [
                            psS.tile([P, 3, SL], F32, name=f"st{j}", tag=f"st{j}")
                            for j in range(2)
                        ]
                        for j in range(2):
                            off = 64 * j
                            for i, kc in enumerate(chunks):
                                nc.tensor.matmul(
                                    sts[j][:, i, xs:SL],
                                    lhsT=kvtK[kc // 8][
                                        off : off + 64, kc % 8, s * P : (s + 1) * P
                                    ],
                                    rhs=qrot_t[s][off : off + 64, xs:SL],
                                    start=True,
                                    stop=True,
                                )
                        expss = []
                        for j in range(2):
                            exps = pe.tile(
                                [P, 3, SL], BF16, name=f"exps{j}", tag=f"exps{j}"
                            )
                            nc.scalar.activation(
                                exps[:, 0:nch, xs:SL],
                                sts[j][:, 0:nch, xs:SL],
                                mybir.ActivationFunctionType.Exp,
                                scale=0.125,
                            )
                            nc.gpsimd.tensor_mul(
                                exps[:, 0:nch, xs : xs + mw],
                                exps[:, 0:nch, xs : xs + mw],
                                mask_sb[:, 0:nch, 0:mw],
                            )
                            expss.append(exps)
                        stash[g] = expss
                        if gi >= 1:
                            emit_avmain(groups[gi - 1])
                        if gi >= 2:
                            emit_avband(groups[gi - 2])
                    emit_avmain(groups[-1])
                    if len(groups) >= 2:
                        emit_avband(groups[-2])
                    emit_avband(groups[-1], last=True)

                    # flush the wave's PSUM partial into the SBUF accumulator
                    for j in range(2):
                        h = 2 * hp + j
                        if w == 0:
                            nc.vector.tensor_copy(acc[:, h, :], ots[j][:])
                        else:
                            nc.vector.tensor_add(
                                acc[:, h, xsw:SL],
                                acc[:, h, xsw:SL],
                                ots[j][:, xsw:SL],
                            )
                        if w == len(WAVES) - 1:
                            den = pn.tile([1, SL], F32, name="den", tag="den")
                            nc.vector.tensor_copy(den[0:1, :], acc[64:65, h, :])
                            recip = pn.tile([1, SL], F32, name="recip", tag="recip")
                            nc.vector.reciprocal(recip[:], den[:])
                            recipb = pn.tile(
                                [64, SL], F32, name="recipb", tag="recipb"
                            )
                            nc.gpsimd.partition_broadcast(recipb[:], recip[:])
                            nc.vector.tensor_mul(
                                osb[:, h, :], acc[0:64, h, :], recipb[:]
                            )

            # ================= Phase C: output projection =================
            if stop_after == "B":
                continue
            with (
                tc.tile_pool(name="pco", bufs=1) as pco,
                tc.tile_pool(name="psC", bufs=1, space="PSUM") as psC,
            ):
                ob_sb = pco.tile([P, KSUB, SL], BF16)
                outps = [
                    psC.tile([P, SL], F32, name=f"outp{m}", tag=f"outp{m}")
                    for m in range(KSUB)
                ]
                # h-major: only the last head-pair's matmuls wait on hp5
                for h in range(H):
                    for m in range(KSUB):
                        nc.tensor.matmul(
                            outps[m][:],
                            lhsT=wo_sb[:, h, m * P : (m + 1) * P],
                            rhs=osb[:, h, :],
                            start=(h == 0),
                            stop=(h == H - 1),
                        )
                for m in range(KSUB):
                    nc.vector.tensor_copy(ob_sb[:, m, :], outps[m][:])
                nc.sync.dma_start(
                    out_d.ap().rearrange("m p n -> p m n"), ob_sb[:]
                )

    nc.compile()
    return nc


def _host_prep(x, position_ids, Wq, Wk, Wv, Wo):
    x2 = np.asarray(x, dtype=np.float32).reshape(S, D)
    pos = np.asarray(position_ids).reshape(S)

    fraction = (2.0 * np.arange(HALF, dtype=np.float32) / DH).astype(np.float32)
    timescale = (10000.0 ** fraction).astype(np.float32)  # [32]

    def tables(p_vec):
        sinu = (p_vec[None, :].astype(np.float32) / timescale[:, None]).astype(
            np.float32
        )
        cos = np.tile(np.cos(sinu).astype(np.float32), (4, 1))
        sin = np.sin(sinu).astype(np.float32)
        # signed for the swap formulation: first-half rows get -sin (they
        # subtract the swapped second half), second-half rows get +sin.
        sin = np.concatenate([-sin, sin, -sin, sin], axis=0)
        return cos, sin

    bf = ml_dtypes.bfloat16
    weights = {
        "wq": np.ascontiguousarray(np.asarray(Wq, dtype=np.float32)).astype(bf),
        "wk": np.ascontiguousarray(np.asarray(Wk, dtype=np.float32)).astype(bf),
        "wv": np.ascontiguousarray(np.asarray(Wv, dtype=np.float32)).astype(bf),
        "wo": np.ascontiguousarray(np.asarray(Wo, dtype=np.float32)).astype(bf),
    }

    in_maps = []
    for c in range(NCORES):
        qrows = np.arange(SL) * NCORES + c
        # kv rows: core c owns global 128-chunks {8j+c}
        kvrows = (
            (np.arange(NQ) * NCORES + c)[:, None] * P + np.arange(P)[None, :]
        ).ravel()
        cosq, sinq = tables(pos[qrows])
        cosk, sink = tables(pos[kvrows])
        trig = np.stack([cosq, sinq, cosk, sink], axis=0).astype(bf)
        pp = np.arange(P)[:, None, None]
        ii = np.arange(3)[None, :, None]
        jj = np.arange(48)[None, None, :]
        mask3 = (P * ii + pp <= NCORES * jj + c).astype(bf)
        m = {
            "xq": np.ascontiguousarray(x2[qrows, :].T).astype(bf),
            "xkv": np.ascontiguousarray(x2[kvrows, :].T).astype(bf),
            "trig": trig,
            "mask3": mask3,
        }
        m.update(weights)
        in_maps.append(m)
    return in_maps


def kernel(x, position_ids, Wq, Wk, Wv, Wo):
    if "nc" not in _cache:
        _cache["nc"] = _build()
    nc = _cache["nc"]
    in_maps = _host_prep(x, position_ids, Wq, Wk, Wv, Wo)
    res = bass_utils.run_bass_kernel_spmd(
        nc, in_maps, core_ids=list(range(NCORES))
    )
    out = np.empty((1, S, D), dtype=np.float32)
    for c in range(NCORES):
        outT = res.results[c]["out"].astype(np.float32).reshape(D, SL)
        out[0, c::NCORES, :] = outT.T
    return out


# revision 16
# speedup vs baseline: 1.5167x; 1.0864x over previous
"""Causal self-attention (B=1, S=4096, D=768, H=12, dh=64) on 8 TRN2 NeuronCores.

Strategy (v3):
  - Sequence-parallel QKV projections + RoPE (each core projects 512 rows).
  - Queries are stride-8 interleaved (core c owns query rows c::8) so causal
    work balances and the program is SPMD-uniform; all per-core variation is
    input data (x slices, rope tables, masks).
  - KV ownership is by interleaved 128-row chunks: core c owns global chunks
    {8j+c}.  AllGather quarter u then delivers chunks 8u..8u+7 in causal
    order AND each rank's contribution is one whole chunk, so the gathered
    quarter reloads into SBUF as ONE DMA per tensor with >=768B runs.
  - K^T is gathered in fp8e4m3 (halves K gather bytes; QK runs as mixed
    fp8 x bf16 matmul), V in bf16 (fp8 V fails the error budget).  V carries
    a ones column per head (memset locally after reload, not gathered) that
    yields the softmax denominator through the AV matmul.
  - Every model input loads with a single strided DMA; K/V-path inputs load
    first so quarter 0 reaches the gather ASAP, Q-path and phase-C inputs
    are deferred behind quarter-0/1 gather traffic (the DMA engine pool is
    the phase-A bottleneck).
  - Attention: transposed layout (keys on partitions via S^T = K^T.T @ Q^T),
    3-chunk groups with a 48-column causal offset; exp runs on the scalar
    engine (one instruction per group x head).  The causal-band mask multiply
    runs on gpsimd, off the exp->AV critical path: group 0 is one full-range
    AV over masked exps carrying the single start=True (a matmul start resets
    the whole PSUM bank), later groups split into a mask-independent main
    region [xs+48, SL) and a 48-wide diagonal band emitted two groups late so
    the Pool latency hides.
  - PE emission order per group: QK(g) first (unblocks the next exp ASAP),
    then AVmain(g-1), then AVband(g-2), keeping the tensor engine dense so
    the p-state stays at max clock.
  - Softmax normalize: DVE reciprocal of the denominator row + gpsimd
    partition-broadcast + DVE multiply, overlapped with the next head-pair.
  - Output projection loops h-major so only the last two heads' matmuls wait
    on the final head-pair; output staged bf16 and widened on the host.
"""

import numpy as np
import ml_dtypes

import concourse.bass as bass
import concourse.bacc as bacc
import concourse.tile as tile
import concourse.mybir as mybir
import concourse.bass_utils as bass_utils

NCORES = 8
S = 4096
D = 768
H = 12
DH = 64
HALF = 32
P = 128
SL = S // NCORES          # 512 local queries / kv rows per core
KSUB = D // P             # 6
NKC = S // P              # 32 key chunks of 128
NQ = 4                    # pipelined AllGather quarters
VW = H * (DH + 1)         # 780: V row width incl. ones col per head
RQK = P * D               # per-rank K elems per quarter
RQV = P * VW              # per-rank V elems per quarter (incl. ones col)
NG3 = (NKC + 2) // 3      # 11 causal groups of <=3 chunks
F32 = mybir.dt.float32
BF16 = mybir.dt.bfloat16
F8 = mybir.dt.float8e4

_cache = {}


def _build(repeats=1, fake_gather=False, stop_after=None):
    nc = bacc.Bacc(
        "TRN2",
        target_bir_lowering=False,
        debug=False,
        enable_asserts=False,
        num_devices=1 if fake_gather else NCORES,
    )
    inp = {}
    for name, shape, dt in [
        ("xq", [D, SL], BF16),
        ("xkv", [D, SL], BF16),
        ("trig", [4, P, SL], BF16),   # cosq, sinq, cosk, sink
        ("mask3", [P, 3, 48], BF16),
        ("wq", [D, D], BF16),
        ("wk", [D, D], BF16),
        ("wv", [D, D], BF16),
        ("wo", [D, D], BF16),
    ]:
        inp[name] = nc.dram_tensor(name, shape, dt, kind="ExternalInput")
    out_d = nc.dram_tensor("out", [KSUB, P, SL], BF16, kind="ExternalOutput")

    with tile.TileContext(nc) as tc:
      for _rep in range(repeats):
        with (
            tc.tile_pool(name="persist", bufs=1) as persist,
            tc.tile_pool(name="dram", bufs=1, space="DRAM") as dram,
        ):
            # ---- persistent tiles ----
            qrot_t = [
                persist.tile([P, SL], BF16, name=f"qrot{s_}", tag=f"qrot{s_}")
                for s_ in range(KSUB)
            ]
            osb = persist.tile([64, H, SL], BF16)
            mask_sb = persist.tile([P, 3, 48], BF16)
            trig_sb = persist.tile([P, 4, SL], BF16)
            wo_sb = persist.tile([64, H, D], BF16)
            kvtK = [
                persist.tile([P, NCORES, D], F8, name=f"kvtK{u}", tag=f"kvtK{u}")
                for u in range(NQ)
            ]
            kvtV = [
                persist.tile(
                    [P, NCORES, H, DH + 1], BF16, name=f"kvtV{u}", tag=f"kvtV{u}"
                )
                for u in range(NQ)
            ]
            # K/V staging lives outside the phase-A pools: a pool close
            # inserts per-engine barriers gated on the staging-write DMAs,
            # which drain the DMA queue late and would stall phase-B work
            # queued behind the barrier.
            kvsK_t = [
                persist.tile([P, D], F8, name=f"kvsK{u}", tag=f"kvsK{u}")
                for u in range(NQ)
            ]
            kvsV_t = [
                persist.tile(
                    [P, H, DH + 1], BF16, name=f"kvsV{u}", tag=f"kvsV{u}"
                )
                for u in range(NQ)
            ]

            kvinK = dram.tile([NQ, RQK], F8)
            kvinV = dram.tile([NQ, RQV], BF16)
            kvoutK = [
                dram.tile(
                    [NCORES, RQK],
                    F8,
                    name=f"kvoutK{u}",
                    addr_space="Local" if fake_gather else "Shared",
                )
                for u in range(NQ)
            ]
            kvoutV = [
                dram.tile(
                    [NCORES, RQV],
                    BF16,
                    name=f"kvoutV{u}",
                    addr_space="Local" if fake_gather else "Shared",
                )
                for u in range(NQ)
            ]

            # ================= Phase A: projections + rope =================
            with (
                tc.tile_pool(name="pw", bufs=1) as pw,
                tc.tile_pool(name="px", bufs=1) as px,
                tc.tile_pool(name="pt", bufs=2) as pt,
                tc.tile_pool(name="psA", bufs=2, space="PSUM") as psA,
            ):
                w_sb = {}
                for name in ["wk", "wv", "wq"]:
                    w_sb[name] = pw.tile([P, KSUB, D], BF16, name=f"{name}_sb")
                xq_sb = px.tile([P, KSUB, SL], BF16)
                xkv_sb = px.tile([P, KSUB, SL], BF16)

                # K/V-path inputs first: the DMA engine pool is the phase-A
                # bottleneck and quarter 0 gates the attention start.
                nc.sync.dma_start(
                    w_sb["wk"][:],
                    inp["wk"].ap().rearrange("(ks p) m -> p ks m", p=P),
                )
                nc.sync.dma_start(
                    xkv_sb[:], inp["xkv"].ap().rearrange("(ks p) n -> p ks n", p=P)
                )
                nc.sync.dma_start(
                    trig_sb[:], inp["trig"].ap().rearrange("t p n -> p t n")
                )
                nc.sync.dma_start(
                    w_sb["wv"][:],
                    inp["wv"].ap().rearrange("(ks p) m -> p ks m", p=P),
                )
                cosq = trig_sb[:, 0, :]
                sinq = trig_sb[:, 1, :]
                cosk = trig_sb[:, 2, :]
                sink = trig_sb[:, 3, :]

                def load_q_inputs():
                    nc.sync.dma_start(
                        w_sb["wq"][:],
                        inp["wq"].ap().rearrange("(ks p) m -> p ks m", p=P),
                    )
                    nc.sync.dma_start(
                        xq_sb[:], inp["xq"].ap().rearrange("(ks p) n -> p ks n", p=P)
                    )

                def load_late_inputs():
                    nc.sync.dma_start(mask_sb[:], inp["mask3"].ap())
                    nc.sync.dma_start(
                        wo_sb[:], inp["wo"].ap().rearrange("(h p) e -> p h e", p=64)
                    )

                def project_rope_q(s):
                    paq = psA.tile([P, SL], F32, name="paq", tag="paq")
                    for ks in range(KSUB):
                        nc.tensor.matmul(
                            paq[:],
                            lhsT=w_sb["wq"][:, ks, s * P : (s + 1) * P],
                            rhs=xq_sb[:, ks, :],
                            start=(ks == 0),
                            stop=(ks == KSUB - 1),
                        )
                    pab = pt.tile([P, SL], BF16, name="pabq", tag="pabq")
                    nc.scalar.copy(pab[:], paq[:])
                    swp = pt.tile([P, SL], BF16, name="swpq", tag="swpq")
                    for (dd, ss2) in [(0, 32), (32, 0), (64, 96), (96, 64)]:
                        nc.vector.tensor_copy(
                            swp[dd : dd + 32, :], pab[ss2 : ss2 + 32, :]
                        )
                    t1 = pt.tile([P, SL], BF16, name="t1q", tag="t1q")
                    t2 = pt.tile([P, SL], BF16, name="t2q", tag="t2q")
                    nc.vector.tensor_mul(t1[:], pab[:], cosq)
                    nc.vector.tensor_mul(t2[:], swp[:], sinq)
                    nc.vector.tensor_add(qrot_t[s][:], t1[:], t2[:])

                for u in range(NQ):
                    kvsK = kvsK_t[u]
                    kvsV = kvsV_t[u]
                    nc.vector.memset(kvsV[:, :, DH : DH + 1], 1.0)
                    for s in range(KSUB):
                        pak = psA.tile([P, P], F32, name="pak", tag="pak")
                        for ks in range(KSUB):
                            nc.tensor.matmul(
                                pak[:],
                                lhsT=w_sb["wk"][:, ks, s * P : (s + 1) * P],
                                rhs=xkv_sb[:, ks, u * P : (u + 1) * P],
                                start=(ks == 0),
                                stop=(ks == KSUB - 1),
                            )
                        pab = pt.tile([P, P], BF16, name="pabk", tag="pabk")
                        nc.scalar.copy(pab[:], pak[:])
                        swp = pt.tile([P, P], BF16, name="swpk", tag="swpk")
                        for (dd, ss2) in [(0, 32), (32, 0), (64, 96), (96, 64)]:
                            nc.vector.tensor_copy(
                                swp[dd : dd + 32, :], pab[ss2 : ss2 + 32, :]
                            )
                        t1 = pt.tile([P, P], BF16, name="t1k", tag="t1k")
                        t2 = pt.tile([P, P], BF16, name="t2k", tag="t2k")
                        nc.vector.tensor_mul(
                            t1[:], pab[:], cosk[:, u * P : (u + 1) * P]
                        )
                        nc.vector.tensor_mul(
                            t2[:], swp[:], sink[:, u * P : (u + 1) * P]
                        )
                        nc.vector.tensor_add(
                            kvsK[:, s * P : (s + 1) * P], t1[:], t2[:]
                        )
                    # j slices are [P, SL] so each stays inside one 2KB PSUM
                    # bank (matmul outputs may not straddle banks)
                    pv = psA.tile([P, 2, SL], F32, name="pv", tag="pv")
                    for j in range(2):
                        for ks in range(KSUB):
                            nc.tensor.matmul(
                                pv[:, j, 0 : D // 2],
                                lhsT=xkv_sb[:, ks, u * P : (u + 1) * P],
                                rhs=w_sb["wv"][:, ks, j * (D // 2) : (j + 1) * (D // 2)],
                                start=(ks == 0),
                                stop=(ks == KSUB - 1),
                            )
                    for j in range(2):
                        nc.scalar.copy(
                            kvsV[:, j * 6 : (j + 1) * 6, 0:DH],
                            pv[:, j, 0 : D // 2].rearrange("p (h d) -> p h d", d=DH),
                        )
                    nc.sync.dma_start(
                        kvinK[u].rearrange("(p x) -> p x", p=P), kvsK[:]
                    )
                    nc.sync.dma_start(
                        kvinV[u].rearrange("(p h d) -> p h d", p=P, h=H), kvsV[:]
                    )
                    if fake_gather:
                        for c in range(NCORES):
                            nc.sync.dma_start(kvoutK[u][c], kvinK[u])
                            nc.sync.dma_start(kvoutV[u][c], kvinV[u])
                    else:
                        nc.gpsimd.collective_compute(
                            "AllGather",
                            mybir.AluOpType.bypass,
                            replica_groups=[list(range(NCORES))],
                            ins=[kvinK[u].opt()],
                            outs=[kvoutK[u][:].opt()],
                        )
                        nc.gpsimd.collective_compute(
                            "AllGather",
                            mybir.AluOpType.bypass,
                            replica_groups=[list(range(NCORES))],
                            ins=[kvinV[u].opt()],
                            outs=[kvoutV[u][:].opt()],
                        )
                    nc.sync.dma_start(
                        kvtK[u][:],
                        kvoutK[u][:].rearrange("c (p x) -> p c x", p=P),
                    )
                    nc.sync.dma_start(
                        kvtV[u][:],
                        kvoutV[u][:].rearrange("c (p h d) -> p c h d", p=P, h=H),
                    )
                    if u == 0:
                        load_q_inputs()
                        project_rope_q(0)
                        project_rope_q(1)
                    elif u == 1:
                        load_late_inputs()
                        project_rope_q(2)
                        project_rope_q(3)
                    elif u == 2:
                        project_rope_q(4)
                        project_rope_q(5)

            # ================= Phase B: attention =================
            # Quarter-aligned waves: wave w covers causal groups whose chunks
            # live in quarters <= w, iterating ALL head-pairs per wave, so
            # attention consumption matches the gather pipeline's delivery
            # rate instead of head-pair 0 burning through every quarter and
            # stalling.  Per-(hp,wave) AV partials accumulate in PSUM and are
            # flushed into an SBUF accumulator by DVE adds.
            if stop_after == "A":
                continue
            WAVES = [[0, 1], [2, 3, 4], [5, 6, 7], [8, 9, 10]]
            with (
                tc.tile_pool(name="pe", bufs=3) as pe,
                tc.tile_pool(name="pn", bufs=2) as pn,
                tc.tile_pool(name="pacc", bufs=1) as pacc,
                tc.tile_pool(name="psS", bufs=1, space="PSUM") as psS,
                tc.tile_pool(name="psO", bufs=1, space="PSUM") as psO,
            ):
                acc = pacc.tile([DH + 1, H, SL], F32)
                for w, groups in enumerate(WAVES):
                  g_min = groups[0]
                  xsw = 48 * g_min
                  for hp in range(H // 2):
                    s = hp
                    ots = [
                        psO.tile([DH + 1, SL], F32, name=f"ot{j}", tag=f"ot{j}")
                        for j in range(2)
                    ]

                    stash = {}

                    # The wave's first group emits one full-range AV over
                    # masked exps carrying the single start=True (a matmul
                    # start resets the whole PSUM bank, so the first-executed
                    # AV must cover the wave's whole column range); later
                    # groups split into a mask-free main region and a 48-wide
                    # diagonal band emitted late so the Pool mask hides.
                    def emit_avmain(g):
                        xs = xsw if g == g_min else 48 * g + 48
                        if xs >= SL:
                            return
                        expss = stash[g]
                        for j in range(2):
                            h = 2 * hp + j
                            for i, kc in enumerate(range(3 * g, min(3 * g + 3, NKC))):
                                nc.tensor.matmul(
                                    ots[j][:, xs:SL],
                                    lhsT=kvtV[kc // 8][:, kc % 8, h, :],
                                    rhs=expss[j][:, i, xs:SL],
                                    start=(g == g_min and i == 0),
                                    stop=False,
                                    skip_group_check=True,
                                )

                    def emit_avband(g, last=False):
                        xs = 48 * g
                        mw = min(48, SL - xs)
                        expss = stash.pop(g)
                        chunks = list(range(3 * g, min(3 * g + 3, NKC)))
                        for j in range(2):
                            h = 2 * hp + j
                            for i, kc in enumerate(chunks):
                                if g == g_min:
                                    continue  # covered by the full-range main
                                nc.tensor.matmul(
                                    ots[j][:, xs : xs + mw],
                                    lhsT=kvtV[kc // 8][:, kc % 8, h, :],
                                    rhs=expss[j][:, i, xs : xs + mw],
                                    start=False,
                                    stop=(last and i == len(chunks) - 1),
                                    skip_group_check=True,
                                )

                    for gi, g in enumerate(groups):
                        chunks = list(range(3 * g, min(3 * g + 3, NKC)))
                        nch = len(chunks)
                        xs = 48 * g
                        mw = min(48, SL - xs)
                        sts = [
                            psS.tile([P, 3, SL], F32, name=f"st{j}", tag=f"st{j}")
                            for j in range(2)
                        ]
                        for j in range(2):
                            off = 64 * j
                            for i, kc in enumerate(chunks):
                                nc.tensor.matmul(
                                    sts[j][:, i, xs:SL],
                                    lhsT=kvtK[kc // 8][
                                        off : off + 64, kc % 8, s * P : (s + 1) * P
                                    ],
                                    rhs=qrot_t[s][off : off + 64, xs:SL],
                                    start=True,
                                    stop=True,
                                )
                        expss = []
                        for j in range(2):
                            exps = pe.tile(
                                [P, 3, SL], BF16, name=f"exps{j}", tag=f"exps{j}"
                            )
                            nc.scalar.activation(
                                exps[:, 0:nch, xs:SL],
                                sts[j][:, 0:nch, xs:SL],
                                mybir.ActivationFunctionType.Exp,
                                scale=0.125,
                            )
                            nc.gpsimd.tensor_mul(
                                exps[:, 0:nch, xs : xs + mw],
                                exps[:, 0:nch, xs : xs + mw],
                                mask_sb[:, 0:nch, 0:mw],
                            )
                            expss.append(exps)
                        stash[g] = expss
                        if gi >= 1:
                            emit_avmain(groups[gi - 1])
                        if gi >= 2:
                            emit_avband(groups[gi - 2])
                    emit_avmain(groups[-1])
                    if len(groups) >= 2:
                        emit_avband(groups[-2])
                    emit_avband(groups[-1], last=True)

                    # flush the wave's PSUM partial into the SBUF accumulator
                    for j in range(2):
                        h = 2 * hp + j
                        if w == 0:
                            nc.vector.tensor_copy(acc[:, h, :], ots[j][:])
                        else:
                            nc.vector.tensor_add(
                                acc[:, h, xsw:SL],
                                acc[:, h, xsw:SL],
                                ots[j][:, xsw:SL],
                            )
                        if w == len(WAVES) - 1:
                            den = pn.tile([1, SL], F32, name="den", tag="den")
                            nc.vector.tensor_copy(den[0:1, :], acc[64:65, h, :])
                            recip = pn.tile([1, SL], F32, name="recip", tag="recip")
                            nc.vector.reciprocal(recip[:], den[:])
                            recipb = pn.tile(
                                [64, SL], F32, name="recipb", tag="recipb"
                            )
                            nc.gpsimd.partition_broadcast(recipb[:], recip[:])
                            nc.vector.tensor_mul(
                                osb[:, h, :], acc[0:64, h, :], recipb[:]
                            )

            # ================= Phase C: output projection =================
            if stop_after == "B":
                continue
            with (
                tc.tile_pool(name="pco", bufs=1) as pco,
                tc.tile_pool(name="psC", bufs=1, space="PSUM") as psC,
            ):
                ob_sb = pco.tile([P, KSUB, SL], BF16)
                outps = [
                    psC.tile([P, SL], F32, name=f"outp{m}", tag=f"outp{m}")
                    for m in range(KSUB)
                ]
                # h-major: only the last head-pair's matmuls wait on hp5
                for h in range(H):
                    for m in range(KSUB):
                        nc.tensor.matmul(
                            outps[m][:],
                            lhsT=wo_sb[:, h, m * P : (m + 1) * P],
                            rhs=osb[:, h, :],
                            start=(h == 0),
                            stop=(h == H - 1),
                        )
                for m in range(KSUB):
                    nc.vector.tensor_copy(ob_sb[:, m, :], outps[m][:])
                nc.sync.dma_start(
                    out_d.ap().rearrange("m p n -> p m n"), ob_sb[:]
                )

    nc.compile()
    return nc


def _host_prep(x, position_ids, Wq, Wk, Wv, Wo):
    x2 = np.asarray(x, dtype=np.float32).reshape(S, D)
    pos = np.asarray(position_ids).reshape(S)

    fraction = (2.0 * np.arange(HALF, dtype=np.float32) / DH).astype(np.float32)
    timescale = (10000.0 ** fraction).astype(np.float32)  # [32]

    def tables(p_vec):
        sinu = (p_vec[None, :].astype(np.float32) / timescale[:, None]).astype(
            np.float32
        )
        cos = np.tile(np.cos(sinu).astype(np.float32), (4, 1))
        sin = np.sin(sinu).astype(np.float32)
        # signed for the swap formulation: first-half rows get -sin (they
        # subtract the swapped second half), second-half rows get +sin.
        sin = np.concatenate([-sin, sin, -sin, sin], axis=0)
        return cos, sin

    bf = ml_dtypes.bfloat16
    weights = {
        "wq": np.ascontiguousarray(np.asarray(Wq, dtype=np.float32)).astype(bf),
        "wk": np.ascontiguousarray(np.asarray(Wk, dtype=np.float32)).astype(bf),
        "wv": np.ascontiguousarray(np.asarray(Wv, dtype=np.float32)).astype(bf),
        "wo": np.ascontiguousarray(np.asarray(Wo, dtype=np.float32)).astype(bf),
    }

    in_maps = []
    for c in range(NCORES):
        qrows = np.arange(SL) * NCORES + c
        # kv rows: core c owns global 128-chunks {8j+c}
        kvrows = (
            (np.arange(NQ) * NCORES + c)[:, None] * P + np.arange(P)[None, :]
        ).ravel()
        cosq, sinq = tables(pos[qrows])
        cosk, sink = tables(pos[kvrows])
        trig = np.stack([cosq, sinq, cosk, sink], axis=0).astype(bf)
        pp = np.arange(P)[:, None, None]
        ii = np.arange(3)[None, :, None]
        jj = np.arange(48)[None, None, :]
        mask3 = (P * ii + pp <= NCORES * jj + c).astype(bf)
        m = {
            "xq": np.ascontiguousarray(x2[qrows, :].T).astype(bf),
            "xkv": np.ascontiguousarray(x2[kvrows, :].T).astype(bf),
            "trig": trig,
            "mask3": mask3,
        }
        m.update(weights)
        in_maps.append(m)
    return in_maps


def kernel(x, position_ids, Wq, Wk, Wv, Wo):
    if "nc" not in _cache:
        _cache["nc"] = _build()
    nc = _cache["nc"]
    in_maps = _host_prep(x, position_ids, Wq, Wk, Wv, Wo)
    res = bass_utils.run_bass_kernel_spmd(
        nc, in_maps, core_ids=list(range(NCORES))
    )
    out = np.empty((1, S, D), dtype=np.float32)
    for c in range(NCORES):
        outT = res.results[c]["out"].astype(np.float32).reshape(D, SL)
        out[0, c::NCORES, :] = outT.T
    return out


# revision 17
# speedup vs baseline: 1.5546x; 1.0250x over previous
"""Causal self-attention (B=1, S=4096, D=768, H=12, dh=64) on 8 TRN2 NeuronCores.

Strategy (v3):
  - Sequence-parallel QKV projections + RoPE (each core projects 512 rows).
  - Queries are stride-8 interleaved (core c owns query rows c::8) so causal
    work balances and the program is SPMD-uniform; all per-core variation is
    input data (x slices, rope tables, masks).
  - KV ownership is by interleaved 128-row chunks: core c owns global chunks
    {8j+c}.  AllGather quarter u then delivers chunks 8u..8u+7 in causal
    order AND each rank's contribution is one whole chunk, so the gathered
    quarter reloads into SBUF as ONE DMA per tensor with >=768B runs.
  - K^T is gathered in fp8e4m3 (halves K gather bytes; QK runs as mixed
    fp8 x bf16 matmul), V in bf16 (fp8 V fails the error budget).  V carries
    a ones column per head (memset locally after reload, not gathered) that
    yields the softmax denominator through the AV matmul.
  - Every model input loads with a single strided DMA; K/V-path inputs load
    first so quarter 0 reaches the gather ASAP, Q-path and phase-C inputs
    are deferred behind quarter-0/1 gather traffic (the DMA engine pool is
    the phase-A bottleneck).
  - Attention: transposed layout (keys on partitions via S^T = K^T.T @ Q^T),
    3-chunk groups with a 48-column causal offset; exp runs on the scalar
    engine (one instruction per group x head).  The causal-band mask multiply
    runs on gpsimd, off the exp->AV critical path: group 0 is one full-range
    AV over masked exps carrying the single start=True (a matmul start resets
    the whole PSUM bank), later groups split into a mask-independent main
    region [xs+48, SL) and a 48-wide diagonal band emitted two groups late so
    the Pool latency hides.
  - PE emission order per group: QK(g) first (unblocks the next exp ASAP),
    then AVmain(g-1), then AVband(g-2), keeping the tensor engine dense so
    the p-state stays at max clock.
  - Softmax normalize: DVE reciprocal of the denominator row + gpsimd
    partition-broadcast + DVE multiply, overlapped with the next head-pair.
  - Output projection loops h-major so only the last two heads' matmuls wait
    on the final head-pair; output staged bf16 and widened on the host.
"""

import numpy as np
import ml_dtypes

import concourse.bass as bass
import concourse.bacc as bacc
import concourse.tile as tile
import concourse.mybir as mybir
import concourse.bass_utils as bass_utils

NCORES = 8
S = 4096
D = 768
H = 12
DH = 64
HALF = 32
P = 128
SL = S // NCORES          # 512 local queries / kv rows per core
KSUB = D // P             # 6
NKC = S // P              # 32 key chunks of 128
NQ = 4                    # pipelined AllGather quarters
VW = H * (DH + 1)         # 780: V row width incl. ones col per head
RQK = P * D               # per-rank K elems per quarter
RQV = P * VW              # per-rank V elems per quarter (incl. ones col)
NG3 = (NKC + 2) // 3      # 11 causal groups of <=3 chunks
F32 = mybir.dt.float32
BF16 = mybir.dt.bfloat16
F8 = mybir.dt.float8e4

_cache = {}


def _build(repeats=1, fake_gather=False, stop_after=None):
    nc = bacc.Bacc(
        "TRN2",
        target_bir_lowering=False,
        debug=False,
        enable_asserts=False,
        num_devices=1 if fake_gather else NCORES,
    )
    inp = {}
    for name, shape, dt in [
        ("xq", [D, SL], BF16),
        ("xkv", [D, SL], BF16),
        ("trig", [4, P, SL], BF16),   # cosq, sinq, cosk, sink
        ("mask3", [P, 3, 48], BF16),
        ("wq", [D, D], BF16),
        ("wk", [D, D], BF16),
        ("wv", [D, D], BF16),
        ("wo", [D, D], BF16),
    ]:
        inp[name] = nc.dram_tensor(name, shape, dt, kind="ExternalInput")
    out_d = nc.dram_tensor("out", [KSUB, P, SL], BF16, kind="ExternalOutput")

    with tile.TileContext(nc) as tc:
      for _rep in range(repeats):
        with (
            tc.tile_pool(name="persist", bufs=1) as persist,
            tc.tile_pool(name="dram", bufs=1, space="DRAM") as dram,
        ):
            # ---- persistent tiles ----
            qrot_t = [
                persist.tile([P, SL], BF16, name=f"qrot{s_}", tag=f"qrot{s_}")
                for s_ in range(KSUB)
            ]
            osb = persist.tile([P, KSUB, SL], BF16)  # head-pair i: head 2i on partitions 0-63, 2i+1 on 64-127
            mask_sb = persist.tile([P, 3, 48], BF16)
            trig_sb = persist.tile([P, 4, SL], BF16)
            wo_sb = persist.tile([P, KSUB, D], BF16)
            kvtK = [
                persist.tile([P, NCORES, D], F8, name=f"kvtK{u}", tag=f"kvtK{u}")
                for u in range(NQ)
            ]
            kvtV = [
                persist.tile(
                    [P, NCORES, H, DH + 1], BF16, name=f"kvtV{u}", tag=f"kvtV{u}"
                )
                for u in range(NQ)
            ]
            # K/V staging lives outside the phase-A pools: a pool close
            # inserts per-engine barriers gated on the staging-write DMAs,
            # which drain the DMA queue late and would stall phase-B work
            # queued behind the barrier.
            kvsK_t = [
                persist.tile([P, D], F8, name=f"kvsK{u}", tag=f"kvsK{u}")
                for u in range(NQ)
            ]
            kvsV_t = [
                persist.tile(
                    [P, H, DH + 1], BF16, name=f"kvsV{u}", tag=f"kvsV{u}"
                )
                for u in range(NQ)
            ]

            kvinK = dram.tile([NQ, RQK], F8)
            kvinV = dram.tile([NQ, RQV], BF16)
            kvoutK = [
                dram.tile(
                    [NCORES, RQK],
                    F8,
                    name=f"kvoutK{u}",
                    addr_space="Local" if fake_gather else "Shared",
                )
                for u in range(NQ)
            ]
            kvoutV = [
                dram.tile(
                    [NCORES, RQV],
                    BF16,
                    name=f"kvoutV{u}",
                    addr_space="Local" if fake_gather else "Shared",
                )
                for u in range(NQ)
            ]

            # ================= Phase A: projections + rope =================
            with (
                tc.tile_pool(name="pw", bufs=1) as pw,
                tc.tile_pool(name="px", bufs=1) as px,
                tc.tile_pool(name="pt", bufs=2) as pt,
                tc.tile_pool(name="psA", bufs=2, space="PSUM") as psA,
            ):
                w_sb = {}
                for name in ["wk", "wv", "wq"]:
                    w_sb[name] = pw.tile([P, KSUB, D], BF16, name=f"{name}_sb")
                xq_sb = px.tile([P, KSUB, SL], BF16)
                xkv_sb = px.tile([P, KSUB, SL], BF16)

                # PE warm-up: ~4.3us of dummy matmuls on a zeroed tile so
                # the p-state clock is at max when the K projection starts
                # (cold matmuls run at 1.2GHz until 3us of continuous work).
                pwarm = pt.tile([P, SL], BF16, name="pwarm", tag="pwarm")
                nc.vector.memset(pwarm[:], 0.0)
                for _wi in range(2):
                    pwp = psA.tile([P, SL], F32, name="paq", tag="paq")
                    for _wj in range(10):
                        nc.tensor.matmul(
                            pwp[:],
                            lhsT=pwarm[:, 0:P],
                            rhs=pwarm[:],
                            start=True,
                            stop=True,
                        )
                # K/V-path inputs first: the DMA engine pool is the phase-A
                # bottleneck and quarter 0 gates the attention start.
                nc.sync.dma_start(
                    w_sb["wk"][:],
                    inp["wk"].ap().rearrange("(ks p) m -> p ks m", p=P),
                )
                nc.sync.dma_start(
                    xkv_sb[:], inp["xkv"].ap().rearrange("(ks p) n -> p ks n", p=P)
                )
                nc.sync.dma_start(
                    trig_sb[:], inp["trig"].ap().rearrange("t p n -> p t n")
                )
                nc.sync.dma_start(
                    w_sb["wv"][:],
                    inp["wv"].ap().rearrange("(ks p) m -> p ks m", p=P),
                )
                cosq = trig_sb[:, 0, :]
                sinq = trig_sb[:, 1, :]
                cosk = trig_sb[:, 2, :]
                sink = trig_sb[:, 3, :]

                def load_q_inputs():
                    nc.sync.dma_start(
                        w_sb["wq"][:],
                        inp["wq"].ap().rearrange("(ks p) m -> p ks m", p=P),
                    )
                    nc.sync.dma_start(
                        xq_sb[:], inp["xq"].ap().rearrange("(ks p) n -> p ks n", p=P)
                    )

                def load_late_inputs():
                    nc.sync.dma_start(mask_sb[:], inp["mask3"].ap())
                    nc.sync.dma_start(
                        wo_sb[:], inp["wo"].ap().rearrange("(i p) e -> p i e", p=P)
                    )

                def project_rope_q(s):
                    paq = psA.tile([P, SL], F32, name="paq", tag="paq")
                    for ks in range(KSUB):
                        nc.tensor.matmul(
                            paq[:],
                            lhsT=w_sb["wq"][:, ks, s * P : (s + 1) * P],
                            rhs=xq_sb[:, ks, :],
                            start=(ks == 0),
                            stop=(ks == KSUB - 1),
                        )
                    pab = pt.tile([P, SL], BF16, name="pabq", tag="pabq")
                    nc.scalar.copy(pab[:], paq[:])
                    swp = pt.tile([P, SL], BF16, name="swpq", tag="swpq")
                    for (dd, ss2) in [(0, 32), (32, 0), (64, 96), (96, 64)]:
                        nc.vector.tensor_copy(
                            swp[dd : dd + 32, :], pab[ss2 : ss2 + 32, :]
                        )
                    t1 = pt.tile([P, SL], BF16, name="t1q", tag="t1q")
                    t2 = pt.tile([P, SL], BF16, name="t2q", tag="t2q")
                    nc.vector.tensor_mul(t1[:], pab[:], cosq)
                    nc.vector.tensor_mul(t2[:], swp[:], sinq)
                    nc.vector.tensor_add(qrot_t[s][:], t1[:], t2[:])

                for u in range(NQ):
                    kvsK = kvsK_t[u]
                    kvsV = kvsV_t[u]
                    nc.vector.memset(kvsV[:, :, DH : DH + 1], 1.0)
                    for s in range(KSUB):
                        pak = psA.tile([P, P], F32, name="pak", tag="pak")
                        for ks in range(KSUB):
                            nc.tensor.matmul(
                                pak[:],
                                lhsT=w_sb["wk"][:, ks, s * P : (s + 1) * P],
                                rhs=xkv_sb[:, ks, u * P : (u + 1) * P],
                                start=(ks == 0),
                                stop=(ks == KSUB - 1),
                            )
                        pab = pt.tile([P, P], BF16, name="pabk", tag="pabk")
                        nc.scalar.copy(pab[:], pak[:])
                        swp = pt.tile([P, P], BF16, name="swpk", tag="swpk")
                        for (dd, ss2) in [(0, 32), (32, 0), (64, 96), (96, 64)]:
                            nc.vector.tensor_copy(
                                swp[dd : dd + 32, :], pab[ss2 : ss2 + 32, :]
                            )
                        t1 = pt.tile([P, P], BF16, name="t1k", tag="t1k")
                        t2 = pt.tile([P, P], BF16, name="t2k", tag="t2k")
                        nc.vector.tensor_mul(
                            t1[:], pab[:], cosk[:, u * P : (u + 1) * P]
                        )
                        nc.vector.tensor_mul(
                            t2[:], swp[:], sink[:, u * P : (u + 1) * P]
                        )
                        nc.vector.tensor_add(
                            kvsK[:, s * P : (s + 1) * P], t1[:], t2[:]
                        )
                    # j slices are [P, SL] so each stays inside one 2KB PSUM
                    # bank (matmul outputs may not straddle banks)
                    pv = psA.tile([P, 2, SL], F32, name="pv", tag="pv")
                    for j in range(2):
                        for ks in range(KSUB):
                            nc.tensor.matmul(
                                pv[:, j, 0 : D // 2],
                                lhsT=xkv_sb[:, ks, u * P : (u + 1) * P],
                                rhs=w_sb["wv"][:, ks, j * (D // 2) : (j + 1) * (D // 2)],
                                start=(ks == 0),
                                stop=(ks == KSUB - 1),
                            )
                    for j in range(2):
                        nc.scalar.copy(
                            kvsV[:, j * 6 : (j + 1) * 6, 0:DH],
                            pv[:, j, 0 : D // 2].rearrange("p (h d) -> p h d", d=DH),
                        )
                    nc.sync.dma_start(
                        kvinK[u].rearrange("(p x) -> p x", p=P), kvsK[:]
                    )
                    nc.sync.dma_start(
                        kvinV[u].rearrange("(p h d) -> p h d", p=P, h=H), kvsV[:]
                    )
                    if fake_gather:
                        for c in range(NCORES):
                            nc.sync.dma_start(kvoutK[u][c], kvinK[u])
                            nc.sync.dma_start(kvoutV[u][c], kvinV[u])
                    else:
                        nc.gpsimd.collective_compute(
                            "AllGather",
                            mybir.AluOpType.bypass,
                            replica_groups=[list(range(NCORES))],
                            ins=[kvinK[u].opt()],
                            outs=[kvoutK[u][:].opt()],
                        )
                        nc.gpsimd.collective_compute(
                            "AllGather",
                            mybir.AluOpType.bypass,
                            replica_groups=[list(range(NCORES))],
                            ins=[kvinV[u].opt()],
                            outs=[kvoutV[u][:].opt()],
                        )
                    nc.sync.dma_start(
                        kvtK[u][:],
                        kvoutK[u][:].rearrange("c (p x) -> p c x", p=P),
                    )
                    nc.sync.dma_start(
                        kvtV[u][:],
                        kvoutV[u][:].rearrange("c (p h d) -> p c h d", p=P, h=H),
                    )
                    if u == 0:
                        load_q_inputs()
                        project_rope_q(0)
                        project_rope_q(1)
                    elif u == 1:
                        load_late_inputs()
                        project_rope_q(2)
                        project_rope_q(3)
                    elif u == 2:
                        project_rope_q(4)
                        project_rope_q(5)

            # ================= Phase B: attention =================
            # Quarter-aligned waves: wave w covers causal groups whose chunks
            # live in quarters <= w, iterating ALL head-pairs per wave, so
            # attention consumption matches the gather pipeline's delivery
            # rate instead of head-pair 0 burning through every quarter and
            # stalling.  Per-(hp,wave) AV partials accumulate in PSUM and are
            # flushed into an SBUF accumulator by DVE adds.
            if stop_after == "A":
                continue
            WAVES = [[0, 1], [2, 3, 4], [5, 6, 7], [8, 9, 10]]
            with (
                tc.tile_pool(name="pe", bufs=3) as pe,
                tc.tile_pool(name="pn", bufs=2) as pn,
                tc.tile_pool(name="pacc", bufs=1) as pacc,
                tc.tile_pool(name="psS", bufs=1, space="PSUM") as psS,
                tc.tile_pool(name="psO", bufs=1, space="PSUM") as psO,
            ):
                acc = pacc.tile([DH + 1, H, SL], F32)
                for w, groups in enumerate(WAVES):
                  g_min = groups[0]
                  xsw = 48 * g_min
                  for hp in range(H // 2):
                    s = hp
                    ots = [
                        psO.tile([DH + 1, SL], F32, name=f"ot{j}", tag=f"ot{j}")
                        for j in range(2)
                    ]

                    stash = {}

                    # The wave's first group emits one full-range AV over
                    # masked exps carrying the single start=True (a matmul
                    # start resets the whole PSUM bank, so the first-executed
                    # AV must cover the wave's whole column range); later
                    # groups split into a mask-free main region and a 48-wide
                    # diagonal band emitted late so the Pool mask hides.
                    def emit_avmain(g):
                        xs = xsw if g == g_min else 48 * g + 48
                        if xs >= SL:
                            return
                        expss = stash[g]
                        for j in range(2):
                            h = 2 * hp + j
                            for i, kc in enumerate(range(3 * g, min(3 * g + 3, NKC))):
                                nc.tensor.matmul(
                                    ots[j][:, xs:SL],
                                    lhsT=kvtV[kc // 8][:, kc % 8, h, :],
                                    rhs=expss[j][:, i, xs:SL],
                                    start=(g == g_min and i == 0),
                                    stop=False,
                                    skip_group_check=True,
                                )

                    def emit_avband(g, last=False):
                        xs = 48 * g
                        mw = min(48, SL - xs)
                        expss = stash.pop(g)
                        chunks = list(range(3 * g, min(3 * g + 3, NKC)))
                        for j in range(2):
                            h = 2 * hp + j
                            for i, kc in enumerate(chunks):
                                if g == g_min:
                                    continue  # covered by the full-range main
                                nc.tensor.matmul(
                                    ots[j][:, xs : xs + mw],
                                    lhsT=kvtV[kc // 8][:, kc % 8, h, :],
                                    rhs=expss[j][:, i, xs : xs + mw],
                                    start=False,
                                    stop=(last and i == len(chunks) - 1),
                                    skip_group_check=True,
                                )

                    for gi, g in enumerate(groups):
                        chunks = list(range(3 * g, min(3 * g + 3, NKC)))
                        nch = len(chunks)
                        xs = 48 * g
                        mw = min(48, SL - xs)
                        sts = [
                            psS.tile([P, 3, SL], F32, name=f"st{j}", tag=f"st{j}")
                            for j in range(2)
                        ]
                        for j in range(2):
                            off = 64 * j
                            for i, kc in enumerate(chunks):
                                nc.tensor.matmul(
                                    sts[j][:, i, xs:SL],
                                    lhsT=kvtK[kc // 8][
                                        off : off + 64, kc % 8, s * P : (s + 1) * P
                                    ],
                                    rhs=qrot_t[s][off : off + 64, xs:SL],
                                    start=True,
                                    stop=True,
                                )
                        expss = []
                        for j in range(2):
                            exps = pe.tile(
                                [P, 3, SL], BF16, name=f"exps{j}", tag=f"exps{j}"
                            )
                            nc.scalar.activation(
                                exps[:, 0:nch, xs:SL],
                                sts[j][:, 0:nch, xs:SL],
                                mybir.ActivationFunctionType.Exp,
                                scale=0.125,
                            )
                            nc.gpsimd.tensor_mul(
                                exps[:, 0:nch, xs : xs + mw],
                                exps[:, 0:nch, xs : xs + mw],
                                mask_sb[:, 0:nch, 0:mw],
                            )
                            expss.append(exps)
                        stash[g] = expss
                        if gi >= 1:
                            emit_avmain(groups[gi - 1])
                        if gi >= 2:
                            emit_avband(groups[gi - 2])
                    emit_avmain(groups[-1])
                    if len(groups) >= 2:
                        emit_avband(groups[-2])
                    emit_avband(groups[-1], last=True)

                    # flush the wave's PSUM partial into the SBUF accumulator
                    for j in range(2):
                        h = 2 * hp + j
                        if w == 0:
                            nc.vector.tensor_copy(acc[:, h, :], ots[j][:])
                        else:
                            nc.vector.tensor_add(
                                acc[:, h, xsw:SL],
                                acc[:, h, xsw:SL],
                                ots[j][:, xsw:SL],
                            )
                        if w == len(WAVES) - 1:
                            den = pn.tile([1, SL], F32, name="den", tag="den")
                            nc.vector.tensor_copy(den[0:1, :], acc[64:65, h, :])
                            recip = pn.tile([1, SL], F32, name="recip", tag="recip")
                            nc.vector.reciprocal(recip[:], den[:])
                            recipb = pn.tile(
                                [64, SL], F32, name="recipb", tag="recipb"
                            )
                            nc.gpsimd.partition_broadcast(recipb[:], recip[:])
                            nc.vector.tensor_mul(
                                osb[64 * j : 64 * j + 64, hp, :],
                                acc[0:64, h, :],
                                recipb[:],
                            )

            # ================= Phase C: output projection =================
            if stop_after == "B":
                continue
            with (
                tc.tile_pool(name="pco", bufs=2) as pco,
                tc.tile_pool(name="psC", bufs=2, space="PSUM") as psC,
            ):
                for m in range(KSUB):
                    outp = psC.tile([P, SL], F32, name="outp", tag="outp")
                    for i in range(KSUB):
                        nc.tensor.matmul(
                            outp[:],
                            lhsT=wo_sb[:, i, m * P : (m + 1) * P],
                            rhs=osb[:, i, :],
                            start=(i == 0),
                            stop=(i == KSUB - 1),
                        )
                    ob = pco.tile([P, SL], BF16, name="ob", tag="ob")
                    nc.vector.tensor_copy(ob[:], outp[:])
                    nc.sync.dma_start(out_d.ap()[m], ob[:])

    nc.compile()
    return nc


def _host_prep(x, position_ids, Wq, Wk, Wv, Wo):
    x2 = np.asarray(x, dtype=np.float32).reshape(S, D)
    pos = np.asarray(position_ids).reshape(S)

    fraction = (2.0 * np.arange(HALF, dtype=np.float32) / DH).astype(np.float32)
    timescale = (10000.0 ** fraction).astype(np.float32)  # [32]

    def tables(p_vec):
        sinu = (p_vec[None, :].astype(np.float32) / timescale[:, None]).astype(
            np.float32
        )
        cos = np.tile(np.cos(sinu).astype(np.float32), (4, 1))
        sin = np.sin(sinu).astype(np.float32)
        # signed for the swap formulation: first-half rows get -sin (they
        # subtract the swapped second half), second-half rows get +sin.
        sin = np.concatenate([-sin, sin, -sin, sin], axis=0)
        return cos, sin

    bf = ml_dtypes.bfloat16
    weights = {
        "wq": np.ascontiguousarray(np.asarray(Wq, dtype=np.float32)).astype(bf),
        "wk": np.ascontiguousarray(np.asarray(Wk, dtype=np.float32)).astype(bf),
        "wv": np.ascontiguousarray(np.asarray(Wv, dtype=np.float32)).astype(bf),
        "wo": np.ascontiguousarray(np.asarray(Wo, dtype=np.float32)).astype(bf),
    }

    in_maps = []
    for c in range(NCORES):
        qrows = np.arange(SL) * NCORES + c
        # kv rows: core c owns global 128-chunks {8j+c}
        kvrows = (
            (np.arange(NQ) * NCORES + c)[:, None] * P + np.arange(P)[None, :]
        ).ravel()
        cosq, sinq = tables(pos[qrows])
        cosk, sink = tables(pos[kvrows])
        trig = np.stack([cosq, sinq, cosk, sink], axis=0).astype(bf)
        pp = np.arange(P)[:, None, None]
        ii = np.arange(3)[None, :, None]
        jj = np.arange(48)[None, None, :]
        mask3 = (P * ii + pp <= NCORES * jj + c).astype(bf)
        m = {
            "xq": np.ascontiguousarray(x2[qrows, :].T).astype(bf),
            "xkv": np.ascontiguousarray(x2[kvrows, :].T).astype(bf),
            "trig": trig,
            "mask3": mask3,
        }
        m.update(weights)
        in_maps.append(m)
    return in_maps


def kernel(x, position_ids, Wq, Wk, Wv, Wo):
    if "nc" not in _cache:
        _cache["nc"] = _build()
    nc = _cache["nc"]
    in_maps = _host_prep(x, position_ids, Wq, Wk, Wv, Wo)
    res = bass_utils.run_bass_kernel_spmd(
        nc, in_maps, core_ids=list(range(NCORES))
    )
    out = np.empty((1, S, D), dtype=np.float32)
    for c in range(NCORES):
        outT = res.results[c]["out"].astype(np.float32).reshape(D, SL)
        out[0, c::NCORES, :] = outT.T
    return out
